# revision 1
# baseline (speedup 1.0000x reference)
"""BestRQ loss kernel for 8 Trainium2 NeuronCores.

Math notes (all exact reformulations of the reference):
  - loss = sum_t m_t * ce_t / (sum(m)*C) with m = pad & masked, C = 1.
  - At every token with m_t = 1, masked_xs_t == mask_emb exactly, so
    logits_t == L0 := mask_emb @ W (one shared [N] row) and
    logsumexp(logits_t) == S0 := logsumexp(L0) (one shared scalar).
    => loss = S0 - (sum_t m_t * L0[target_t]) / sum(m).
    The whole [B,T,N] logits tensor / softmax is unnecessary.
  - target_t = argmin_n dist = argmax_n score_tn,
    score_tn = proj_t . emb_n - 0.5*|emb_n|^2.
  - L0[target_t] is extracted without computing any argmax index:
        maxs_t = max_n score_tn            (K=32 matmul)
        maxv_t = max_n (score_tn + delta*L0_n)   (K=33 matmul, same prefix)
        L0[target_t] ~= (maxv_t - maxs_t) / delta
    Both matmuls share the same fp32 accumulation prefix over rows 0..31,
    so the subtraction is Sterbenz-exact up to one ulp of score.
  - Score matmul inputs are bf16 (PE fp32 runs LOW_HIGH at half rate and
    was HAM-throttled); the resulting ~0.03 absolute score noise flips
    near-tied argmaxes for a few % of tokens, but each flip substitutes a
    near-equivalent codeword whose L0 differs by ~0.05 with random sign,
    and the loss averages ~4096 tokens -> ~2e-5 relative error.
  - Only masked tokens matter, so the host gathers the ~4096 masked token
    positions, splits them across the 8 cores and pads to a static shape.
    Padded slots carry m=0 and contribute exactly zero.
"""

import numpy as np

try:
    import concourse.bass as bass  # noqa: F401
except ImportError:  # pragma: no cover
    import sys

    sys.path.insert(0, "/opt/trn_rl_repo")
    import concourse.bass as bass  # noqa: F401

import concourse.mybir as mybir
from concourse import bacc, bass_utils, masks
from concourse.tile import TileContext

F32 = mybir.dt.float32
BF16 = mybir.dt.bfloat16
U8 = mybir.dt.uint8

B, T, D, E, N = 16, 512, 256, 16, 8192
NCORES = 8
EPS = 1e-5
DELTA = 1e-2

NT = 5          # token tiles per core (5*128 = 640 slots >= worst-case masked count)
TOK = NT * 128
BLK = 1024      # psum score-block width (2 banks, one bf16 matmul)
NBLK = N // BLK
BETA = 2000.0   # sharpness of the exp-moment argmax extraction

_CACHE = {}


def _build_bass():
    nc = bacc.Bacc(
        "TRN2", target_bir_lowering=False, debug=False, num_devices=NCORES
    )
    xs = nc.dram_tensor("xs", [TOK, D], F32, kind="ExternalInput")
    pm = nc.dram_tensor("pm", [TOK], U8, kind="ExternalInput")
    mm = nc.dram_tensor("mm", [TOK], U8, kind="ExternalInput")
    gamma = nc.dram_tensor("gamma", [D], F32, kind="ExternalInput")
    beta = nc.dram_tensor("beta", [D], F32, kind="ExternalInput")
    projw = nc.dram_tensor("projw", [D, E], F32, kind="ExternalInput")
    emb = nc.dram_tensor("emb", [E, N], F32, kind="ExternalInput")
    wmat = nc.dram_tensor("wmat", [D, N], F32, kind="ExternalInput")
    maske = nc.dram_tensor("maske", [D], F32, kind="ExternalInput")
    out = nc.dram_tensor("out", [3, 1], F32, kind="ExternalOutput")

    AX = mybir.AxisListType.X
    OP = mybir.AluOpType
    AF = mybir.ActivationFunctionType

    with TileContext(nc) as tc:
        with (
            tc.tile_pool(name="const", bufs=1) as cst,
            tc.tile_pool(name="embp", bufs=1) as embp,
            tc.tile_pool(name="wp", bufs=4) as wp,
            tc.tile_pool(name="xsp", bufs=2) as xsp,
            tc.tile_pool(name="work", bufs=2) as wk,
            tc.tile_pool(name="small", bufs=4) as sm,
            tc.tile_pool(name="vsb", bufs=3) as vsbp,
            tc.tile_pool(name="psc", bufs=3, space="PSUM") as psc,
            tc.tile_pool(name="psm", bufs=2, space="PSUM") as psm,
        ):
            # ---------------- constants / setup ----------------
            ident = cst.tile([128, 128], F32)
            masks.make_identity(nc, ident[:])

            ones128 = cst.tile([128, 1], F32)
            nc.vector.memset(ones128[:], 1.0)

            scale2 = cst.tile([2, 1], F32)
            nc.vector.memset(scale2[:], 1.0)
            nc.vector.tensor_scalar(
                scale2[0:1, :], scale2[0:1, :], 1.0 / (DELTA * BETA), None,
                op0=OP.mult,
            )

            epsb = cst.tile([128, 1], F32)
            nc.vector.memset(epsb[:], EPS)

            # gamma/beta/mask_emb as two 128-row chunks
            gam = cst.tile([128, 2], F32)
            bet = cst.tile([128, 2], F32)
            mke = cst.tile([128, 2], F32)
            nc.sync.dma_start(gam[:], gamma.rearrange("(a b) -> b a", b=128))
            nc.sync.dma_start(bet[:], beta.rearrange("(a b) -> b a", b=128))
            nc.sync.dma_start(mke[:], maske.rearrange("(a b) -> b a", b=128))

            # projection, gamma-folded: Pp[:, kc, :] = gamma_chunk * P_chunk
            praw = cst.tile([128, 2, E], F32)
            nc.sync.dma_start(praw[:], projw.rearrange("(a b) e -> b a e", b=128))
            pp = cst.tile([128, 2, E], F32)
            for kc in range(2):
                nc.vector.tensor_scalar(
                    pp[:, kc, :], praw[:, kc, :], gam[:, kc : kc + 1], None,
                    op0=OP.mult,
                )

            # b0T = Pp^T beta  [E,1]
            b0ps = psm.tile([E, 1], F32, tag="misc")
            for kc in range(2):
                nc.tensor.matmul(
                    b0ps[:], pp[:, kc, :], bet[:, kc : kc + 1],
                    start=(kc == 0), stop=(kc == 1),
                )
            b0t = cst.tile([E, 1], F32)
            nc.vector.tensor_copy(b0t[:], b0ps[:])

            # em3b (bf16): rows 0:16 emb, 16:32 emb^2, 32 delta*L0
            em3f = embp.tile([16, N], F32)
            nc.sync.dma_start(em3f[:], emb[:, :])
            em3b = embp.tile([33, N], BF16)
            nc.vector.tensor_copy(em3b[0:16, :], em3f[:])
            sq16 = embp.tile([16, N], BF16)
            nc.scalar.activation(sq16[:], em3f[:], AF.Square)
            nc.sync.dma_start(em3b[16:32, :], sq16[:])

            # delta*L0 row via W stream (fp32); matmul lands on partition 32
            for ncx in range(16):
                sl = slice(ncx * 512, (ncx + 1) * 512)
                l0ps = psm.tile([33, 512], F32, tag="misc")
                for kc in range(2):
                    wt = wp.tile([128, 512], F32)
                    nc.sync.dma_start(
                        wt[:], wmat[kc * 128 : (kc + 1) * 128, sl]
                    )
                    nc.tensor.matmul(
                        l0ps[32:33, :], mke[:, kc : kc + 1], wt[:],
                        start=(kc == 0), stop=(kc == 1),
                    )
                nc.scalar.activation(
                    em3b[32:33, sl], l0ps[32:33, :], AF.Copy, scale=DELTA
                )

            # S0 = log(sum(exp(L0)))  (L0 tiny => no max subtraction needed)
            etrash = embp.tile([33, N], BF16)
            acc33 = cst.tile([33, 1], F32)
            nc.scalar.activation(
                etrash[32:33, :], em3b[32:33, :], AF.Exp,
                scale=1.0 / DELTA, accum_out=acc33[32:33, :],
            )
            s0t = cst.tile([33, 1], F32)
            nc.scalar.activation(s0t[32:33, :], acc33[32:33, :], AF.Ln)

            # masks -> m_sb [128, NT] fp32
            pm8 = sm.tile([128, NT], U8)
            mm8 = sm.tile([128, NT], U8)
            nc.sync.dma_start(pm8[:], pm.rearrange("(a b) -> b a", b=128))
            nc.sync.dma_start(mm8[:], mm.rearrange("(a b) -> b a", b=128))
            pmf = sm.tile([128, NT], F32)
            mmf = sm.tile([128, NT], F32)
            nc.vector.tensor_copy(pmf[:], pm8[:])
            nc.vector.tensor_copy(mmf[:], mm8[:])
            m_sb = cst.tile([128, NT], F32)
            nc.vector.tensor_tensor(m_sb[:], pmf[:], mmf[:], op=OP.mult)

            numacc = cst.tile([128, NT], F32)

            # ---------------- per token-tile main loop ----------------
            for i in range(NT):
                x_t = xsp.tile([128, D], F32)
                nc.sync.dma_start(x_t[:], xs[i * 128 : (i + 1) * 128, :])

                ssum = sm.tile([128, 1], F32)
                nc.vector.tensor_reduce(ssum[:], x_t[:], axis=AX, op=OP.add)
                mu = sm.tile([128, 1], F32)
                nc.vector.tensor_scalar(mu[:], ssum[:], 1.0 / D, None, op0=OP.mult)
                xc = wk.tile([128, D], F32)
                nc.vector.tensor_scalar(xc[:], x_t[:], mu[:], None, op0=OP.subtract)

                sqt = wk.tile([128, D], F32)
                ssq = sm.tile([128, 1], F32)
                nc.scalar.activation(sqt[:], xc[:], AF.Square, accum_out=ssq[:])
                # rstd = exp(-0.5*ln(var+eps)) — keeps ACT inside the
                # natural_log_exp table set (Sqrt would thrash table loads)
                lnv = sm.tile([128, 1], F32)
                nc.scalar.activation(
                    lnv[:], ssq[:], AF.Ln, scale=1.0 / D, bias=epsb[:]
                )
                rstd = sm.tile([128, 1], F32)
                nc.scalar.activation(rstd[:], lnv[:], AF.Exp, scale=-0.5)
                z = wk.tile([128, D], F32)
                nc.vector.tensor_scalar(z[:], xc[:], rstd[:], None, op0=OP.mult)

                # zT (D on partitions) via PE transpose
                zt = wk.tile([128, 2, 128], F32)
                for kc in range(2):
                    tp = psm.tile([128, 128], F32, tag="misc")
                    nc.tensor.transpose(
                        tp[:], z[:, kc * 128 : (kc + 1) * 128], ident[:]
                    )
                    nc.vector.tensor_copy(zt[:, kc, :], tp[:])

                # projT [E, 128] + bias b0t; build bf16 lhsT33
                ppj = psm.tile([E, 128], F32, tag="misc")
                for kc in range(2):
                    nc.tensor.matmul(
                        ppj[:], pp[:, kc, :], zt[:, kc, :],
                        start=(kc == 0), stop=(kc == 1),
                    )
                lhs = wk.tile([33, 128], BF16)
                nc.vector.memset(lhs[0:32, :], -0.5)
                nc.vector.tensor_scalar(
                    lhs[0:E, :], ppj[:], b0t[:], None, op0=OP.add
                )
                nc.vector.memset(lhs[32:33, :], 1.0)

                maxs_c = sm.tile([128, NBLK], F32)
                vsum_c = sm.tile([128, NBLK], F32)

                # phase A: score matmuls, DVE max-scan straight from psum
                for g in range(NBLK):
                    pa = psc.tile([128, BLK], F32, tag="blk")
                    for h in range(BLK // 512):
                        sl = slice(g * BLK + h * 512, g * BLK + (h + 1) * 512)
                        nc.tensor.matmul(
                            pa[:, h * 512 : (h + 1) * 512],
                            lhs[0:32, :], em3b[0:32, sl],
                            start=True, stop=True,
                        )
                    nc.vector.tensor_reduce(
                        maxs_c[:, g : g + 1], pa[:], axis=AX, op=OP.max
                    )
                maxs = sm.tile([128, 1], F32)
                nc.vector.tensor_reduce(maxs[:], maxs_c[:], axis=AX, op=OP.max)
                nbm = sm.tile([128, 1], F32)
                nc.vector.tensor_scalar(
                    nbm[:], maxs[:], -BETA, None, op0=OP.mult
                )

                # phase B: v matmuls; ACT does exp(beta*(v-maxs)) + sum-accum
                # straight from psum. ln(sum) ~= beta*delta*L0[argmax score].
                for g in range(NBLK):
                    pb = psc.tile([128, BLK], F32, tag="blk")
                    for h in range(BLK // 512):
                        sl = slice(g * BLK + h * 512, g * BLK + (h + 1) * 512)
                        nc.tensor.matmul(
                            pb[:, h * 512 : (h + 1) * 512],
                            lhs[0:33, :], em3b[0:33, sl],
                            start=True, stop=True,
                        )
                    etr = vsbp.tile([128, BLK], BF16)
                    nc.scalar.activation(
                        etr[:], pb[:], AF.Exp, scale=BETA, bias=nbm[:],
                        accum_out=vsum_c[:, g : g + 1],
                    )

                vsum = sm.tile([128, 1], F32)
                nc.vector.tensor_reduce(vsum[:], vsum_c[:], axis=AX, op=OP.add)
                dl0 = sm.tile([128, 1], F32)
                nc.scalar.activation(dl0[:], vsum[:], AF.Ln)
                nc.vector.tensor_tensor(
                    numacc[:, i : i + 1], dl0[:], m_sb[:, i : i + 1], op=OP.mult
                )

            # ---------------- finalize ----------------
            pair = cst.tile([128, 2], F32)
            nc.vector.tensor_reduce(pair[:, 0:1], numacc[:], axis=AX, op=OP.add)
            nc.vector.tensor_reduce(pair[:, 1:2], m_sb[:], axis=AX, op=OP.add)
            pps = psm.tile([2, 1], F32, tag="misc")
            nc.tensor.matmul(pps[:], pair[:], ones128[:], start=True, stop=True)
            pout = cst.tile([2, 1], F32)
            nc.vector.tensor_scalar(pout[:], pps[:], scale2[:], None, op0=OP.mult)
            nc.sync.dma_start(out[0:2, :], pout[:])
            nc.sync.dma_start(out[2:3, :], s0t[32:33, :])

    nc.finalize()
    return nc


def _prep_in_maps(xs, pad_mask, masked_masks, ln_gamma, ln_beta, projection,
                  embeddings, top_n_out, mask_emb):
    xsf = np.ascontiguousarray(np.asarray(xs, np.float32).reshape(B * T, D))
    pmf = np.asarray(pad_mask).reshape(-1).astype(bool)
    mmf = np.asarray(masked_masks).reshape(-1).astype(bool)

    shared = {
        "gamma": np.ascontiguousarray(np.asarray(ln_gamma, np.float32)),
        "beta": np.ascontiguousarray(np.asarray(ln_beta, np.float32)),
        "projw": np.ascontiguousarray(np.asarray(projection, np.float32)),
        "emb": np.ascontiguousarray(np.asarray(embeddings, np.float32)[0]),
        "wmat": np.ascontiguousarray(np.asarray(top_n_out, np.float32)[0]),
        "maske": np.ascontiguousarray(np.asarray(mask_emb, np.float32)),
    }

    # only tokens with pad & masked contribute; gather and spread across cores
    sel = np.nonzero(pmf & mmf)[0]
    assert len(sel) <= NCORES * TOK, (
        f"masked token count {len(sel)} exceeds static capacity {NCORES * TOK}"
    )
    chunks = np.array_split(sel, NCORES)
    in_maps = []
    for c in range(NCORES):
        idx = chunks[c]
        n = len(idx)
        xs_c = np.zeros((TOK, D), np.float32)
        pm_c = np.zeros((TOK,), np.uint8)
        mm_c = np.zeros((TOK,), np.uint8)
        if n:
            xs_c[:n] = xsf[idx]
            pm_c[:n] = pmf[idx]
            mm_c[:n] = mmf[idx]
        in_maps.append({"xs": xs_c, "pm": pm_c, "mm": mm_c, **shared})
    return in_maps


def kernel(**inputs) -> np.ndarray:
    if "nc" not in _CACHE:
        _CACHE["nc"] = _build_bass()
    nc = _CACHE["nc"]
    in_maps = _prep_in_maps(**inputs)
    res = bass_utils.run_bass_kernel_spmd(nc, in_maps, core_ids=list(range(NCORES)))
    num = 0.0
    cnt = 0.0
    s0 = None
    for r in res.results:
        o = r["out"].reshape(3)
        num += float(o[0])
        cnt += float(o[1])
        s0 = float(o[2])
    loss = np.float32(s0 - num / cnt)
    return np.asarray(loss, np.float32)



# revision 4
# speedup vs baseline: 1.3661x; 1.3661x over previous
"""BestRQ loss kernel for 8 Trainium2 NeuronCores.

Math (exact reformulations of the reference):
  - loss = sum_t m_t*ce_t / (sum(m)*C), m = pad & masked, C = 1.
  - At masked tokens, masked_xs == mask_emb exactly, so logits_t == L0 :=
    mask_emb @ W (one shared [N] row), logsumexp(logits_t) == S0.
    => loss = S0 - (sum_t m_t * L0[target_t]) / sum(m).
  - target_t = argmax_n score_tn, score_tn = proj_t . emb_n - 0.5*|emb_n|^2.
  - L0[target_t] extracted without an argmax index:
        maxs_t = max_n score_tn                       (K=32 stream, DVE max)
        ln sum_n exp(beta*(score_tn + delta*L0_n - maxs_t)) ~= beta*delta*L0[target_t]
    (the K=33 stream adds a delta*L0 row; beta=2000 makes the softmax a
    near-exact argmax selector; near-ties contribute noise orders of
    magnitude below the loss scale).
  - Only masked tokens matter: host gathers them, 512/core on 8 cores
    (4 tiles of 128); the handful of leftover tokens (masked count mod
    4096) are folded in exactly on the host - they are <0.5% of the sum.

Engine layout per 128-token tile (all matmuls bf16):
  PE   : score stream K=32 at row-strip 0 and score+dL0 stream K=33 at
         row 64 (tile_position row packing -> the two streams execute
         concurrently), plus the small projection matmuls.
  DVE  : 8x max-reduce of [128,1024] psum blocks (the pacer), LN stats.
  ACT  : 8x exp+accum of [128,1024] psum blocks, rstd via Ln/Exp
         (single activation-table set), per-tile Ln.
  DMA  : xs/emb/W streams, z transposes via the xbar, L0 round-trip.
  L0 = mask_emb @ W computed on 4 psum partition rows (0/32/64/96) so the
  S0 logsumexp partials and the delta*L0 extraction run 4-way parallel;
  a DRAM round-trip + gpsimd cast-DMA plants the delta*L0 row for the
  K=33 stream.
"""

import numpy as np

try:
    import concourse.bass as bass  # noqa: F401
except ImportError:  # pragma: no cover
    import sys

    sys.path.insert(0, "/opt/trn_rl_repo")
    import concourse.bass as bass  # noqa: F401

import concourse.mybir as mybir
from concourse import bacc, bass_utils
from concourse.tile import TileContext

F32 = mybir.dt.float32
BF16 = mybir.dt.bfloat16
U8 = mybir.dt.uint8
NP_BF16 = mybir.dt.np(BF16)

B, T, D, E, N = 16, 512, 256, 16, 8192
NCORES = 8
EPS = 1e-5
DELTA = 1e-2
BETA = 2000.0

NT = 4          # token tiles per core
TOK = NT * 128  # 512 device tokens per core; leftovers go to the host
BLK = 1024      # psum block width (2 banks)
NBLK = N // BLK

_CACHE = {}


def _build_bass():
    nc = bacc.Bacc(
        "TRN2", target_bir_lowering=False, debug=False, num_devices=NCORES
    )
    xs = nc.dram_tensor("xs", [TOK, D], F32, kind="ExternalInput")
    msk = nc.dram_tensor("msk", [TOK], U8, kind="ExternalInput")
    emt = nc.dram_tensor("emt", [32, N], BF16, kind="ExternalInput")
    wmat = nc.dram_tensor("wmat", [128, 2, N], BF16, kind="ExternalInput")
    mke = nc.dram_tensor("mke", [128, 2], BF16, kind="ExternalInput")
    ppw = nc.dram_tensor("ppw", [128, 2, E], BF16, kind="ExternalInput")
    b0v = nc.dram_tensor("b0v", [16, 1], F32, kind="ExternalInput")
    out = nc.dram_tensor("out", [9, 1], F32, kind="ExternalOutput")
    l0scr = nc.dram_tensor("l0scr", [2, 4, BLK], F32, kind="Internal")

    AX = mybir.AxisListType.X
    OP = mybir.AluOpType
    AF = mybir.ActivationFunctionType

    with TileContext(nc) as tc:
        with (
            tc.tile_pool(name="cst", bufs=1) as cst,
            tc.tile_pool(name="wstg", bufs=2) as wstg,
            tc.tile_pool(name="xsp", bufs=NT) as xsp,
            tc.tile_pool(name="wk", bufs=2) as wk,
            tc.tile_pool(name="sm", bufs=2) as sm,
            tc.tile_pool(name="psa", bufs=2, space="PSUM") as psa,
            tc.tile_pool(name="psb", bufs=2, space="PSUM") as psb,
        ):
            # ---------------- constants ----------------
            epsb = cst.tile([128, 1], F32)
            nc.vector.memset(epsb[:], EPS)
            ones128 = cst.tile([128, 1], F32)
            nc.vector.memset(ones128[:], 1.0)

            # masks -> m_sb [128, NT]
            msk8 = cst.tile([128, NT], U8)
            nc.sync.dma_start(msk8[:], msk.rearrange("(a b) -> b a", b=128))
            m_sb = cst.tile([128, NT], F32)
            nc.vector.tensor_copy(m_sb[:], msk8[:])

            # em3b: rows 0:16 emb, 16:32 emb^2, 32 delta*L0 (filled later);
            # duplicated at rows 64:96 + 96 for the row-packed K=33 stream.
            em3b = cst.tile([128, N], BF16)
            nc.sync.dma_start(em3b[0:32, :], emt[:, :])
            nc.sync.dma_start(em3b[64:96, :], emt[:, :])

            # projection weights (gamma-folded on host), proj bias, mask emb
            ppw_sb = cst.tile([128, 2, E], BF16)
            nc.sync.dma_start(ppw_sb[:], ppw[:, :, :])
            mke_sb = cst.tile([128, 2], BF16)
            nc.sync.dma_start(mke_sb[:], mke[:, :])
            b0t = cst.tile([128, 1], F32)
            nc.sync.dma_start(b0t[0:16, :], b0v[:, :])
            nc.sync.dma_start(b0t[64:80, :], b0v[:, :])

            # two static lhs tiles (manual double-buffer): constant rows
            # memset once; per-tile only rows 0:16 / 64:80 are rewritten.
            lhs_t = []
            for li in range(2):
                lh = cst.tile([128, 128], BF16, name=f"lhs{li}")
                nc.vector.memset(lh[:], 0.0)
                # rows 0:16 / 64:80 hold -0.5 only until the first per-tile
                # projection write lands (32-aligned partition bases only)
                nc.vector.memset(lh[0:32, :], -0.5)
                nc.vector.memset(lh[64:96, :], -0.5)
                nc.vector.memset(lh[96:97, :], 1.0)
                lhs_t.append(lh)

            # xs tiles up front (cheap, keeps the DMA queue ahead)
            x_t = []
            for i in range(NT):
                xt = xsp.tile([128, D], F32, name=f"xt{i}")
                nc.sync.dma_start(xt[:], xs[i * 128 : (i + 1) * 128, :])
                x_t.append(xt)

            s0p = cst.tile([128, 2], F32)

            # ---------------- W stream -> L0 rows + S0 partials ----------
            # psum slot s holds L0 for codes [4096s, 4096s+4096) on
            # partition rows {0,32,64,96} x 1024 columns.
            for s in range(2):
                psl = psb.tile([128, BLK], F32, tag="blk", name=f"psl{s}")
                for j in range(4):
                    c = 4 * s + j
                    wt = wstg.tile([128, 2, BLK], BF16, name=f"wt{c}")
                    eng = nc.sync if c % 2 == 0 else nc.scalar
                    eng.dma_start(wt[:], wmat[:, :, c * BLK : (c + 1) * BLK])
                    r = 32 * j
                    for h in range(2):
                        sl = slice(h * 512, (h + 1) * 512)
                        for kc in range(2):
                            nc.tensor.matmul(
                                psl[r : r + 1, sl],
                                mke_sb[:, kc : kc + 1],
                                wt[:, kc, sl],
                                start=(kc == 0), stop=(kc == 1),
                                tile_position=(0, r),
                            )
                # S0 partials: exp(L0) summed per partition; only rows
                # {0,32,64,96} carry data (others hold harmless garbage -
                # engine APs cannot stride partitions, DMAs below can).
                strash = wk.tile([128, BLK], BF16, tag="strash", bufs=1)
                nc.scalar.activation(
                    strash[:], psl[:], AF.Exp,
                    accum_out=s0p[:, s : s + 1],
                )
                # delta*L0 -> sbuf f32 -> dram
                l0sb = wk.tile([128, BLK], F32, tag="l0sb", name=f"l0sb{s}")
                nc.vector.tensor_scalar(
                    l0sb[:], psl[:], DELTA, None, op0=OP.mult
                )
                nc.sync.dma_start(l0scr[s, :, :], l0sb[0:97:32, :])
            # dram -> bf16 row 32 (+ copy at 96) via gpsimd casting DMA
            nc.gpsimd.dma_start(em3b[32:33, :], l0scr[:, :, :])
            nc.gpsimd.dma_start(em3b[96:97, :], l0scr[:, :, :])

            numacc = cst.tile([128, NT], F32)

            # ---------------- per token-tile main loop ----------------
            for i in range(NT):
                lhs = lhs_t[i % 2]
                x = x_t[i]

                # layer norm stats (biased var) via bn_stats
                stats = wk.tile([128, 6], F32, tag="stats")
                nc.vector.bn_stats(stats[:], x[:])
                mv = wk.tile([128, 2], F32, tag="mv")
                nc.vector.bn_aggr(mv[:], stats[:])
                # rstd = exp(-0.5*ln(var+eps))  (stays in natural_log_exp set)
                lnv = sm.tile([128, 1], F32, tag="lnv")
                nc.scalar.activation(lnv[:], mv[:, 1:2], AF.Ln, bias=epsb[:])
                rstd = sm.tile([128, 1], F32, tag="rstd")
                nc.scalar.activation(rstd[:], lnv[:], AF.Exp, scale=-0.5)
                bt = sm.tile([128, 1], F32, tag="bt")
                nc.vector.tensor_scalar(
                    bt[:], mv[:, 0:1], rstd[:], -1.0, op0=OP.mult, op1=OP.mult
                )
                z = wk.tile([128, D], BF16, tag="z")
                nc.vector.tensor_scalar(
                    z[:], x[:], rstd[:], bt[:], op0=OP.mult, op1=OP.add
                )

                # zT via DMA xbar transpose (bf16)
                zt = wk.tile([128, 2, 128], BF16, tag="zt")
                for kc in range(2):
                    nc.sync.dma_start(
                        zt[:, kc, :], z[:, kc * 128 : (kc + 1) * 128],
                        transpose=True,
                    )

                # projT at psum rows 0:16 and 64:80
                ppj = psa.tile([128, 128], F32, tag="blk", name=f"ppj{i}")
                for pos in (0, 64):
                    for kc in range(2):
                        nc.tensor.matmul(
                            ppj[pos : pos + 16, :],
                            ppw_sb[:, kc, :], zt[:, kc, :],
                            start=(kc == 0), stop=(kc == 1),
                            tile_position=(0, pos),
                        )
                nc.vector.tensor_scalar(
                    lhs[0:16, :], ppj[0:16, :], b0t[0:16, :], None, op0=OP.add
                )
                nc.vector.tensor_scalar(
                    lhs[64:80, :], ppj[64:80, :], b0t[64:80, :], None,
                    op0=OP.add,
                )

                maxs_c = sm.tile([128, NBLK], F32, tag="maxc")
                vsum_c = sm.tile([128, NBLK], F32, tag="vsumc")

                # phase A: score matmuls (K=32, strip 0) + DVE max
                for g in range(NBLK):
                    pa = psa.tile([128, BLK], F32, tag="blk", name=f"pa{i}_{g}")
                    for h in range(2):
                        sl = slice(g * BLK + h * 512, g * BLK + (h + 1) * 512)
                        nc.tensor.matmul(
                            pa[:, h * 512 : (h + 1) * 512],
                            lhs[0:32, :], em3b[0:32, sl],
                            start=True, stop=True, tile_position=(0, 0),
                        )
                    nc.vector.tensor_reduce(
                        maxs_c[:, g : g + 1], pa[:], axis=AX, op=OP.max
                    )
                maxs = sm.tile([128, 1], F32, tag="maxs")
                nc.vector.tensor_reduce(maxs[:], maxs_c[:], axis=AX, op=OP.max)
                nbm = sm.tile([128, 1], F32, tag="nbm")
                nc.vector.tensor_scalar(
                    nbm[:], maxs[:], -BETA, None, op0=OP.mult
                )

                # phase B: score+dL0 matmuls (K=33, rows 64:97) + ACT exp
                for g in range(NBLK):
                    pb = psb.tile([128, BLK], F32, tag="blk", name=f"pb{i}_{g}")
                    for h in range(2):
                        sl = slice(g * BLK + h * 512, g * BLK + (h + 1) * 512)
                        nc.tensor.matmul(
                            pb[:, h * 512 : (h + 1) * 512],
                            lhs[64:97, :], em3b[64:97, sl],
                            start=True, stop=True, tile_position=(64, 0),
                        )
                    btrash = wk.tile([128, BLK], BF16, tag="btrash", bufs=1)
                    nc.scalar.activation(
                        btrash[:], pb[:], AF.Exp, scale=BETA, bias=nbm[:],
                        accum_out=vsum_c[:, g : g + 1],
                    )

                vsum = sm.tile([128, 1], F32, tag="vsum")
                nc.vector.tensor_reduce(vsum[:], vsum_c[:], axis=AX, op=OP.add)
                dl0 = sm.tile([128, 1], F32, tag="dl0")
                nc.scalar.activation(dl0[:], vsum[:], AF.Ln)
                nc.vector.tensor_tensor(
                    numacc[:, i : i + 1], dl0[:], m_sb[:, i : i + 1],
                    op=OP.mult,
                )

            # ---------------- finalize ----------------
            numcol = cst.tile([128, 1], F32)
            nc.vector.tensor_reduce(numcol[:], numacc[:], axis=AX, op=OP.add)
            ps2 = psa.tile([128, 1], F32, tag="blk", name="ps2")
            nc.tensor.matmul(
                ps2[0:1, :], numcol[:], ones128[:], start=True, stop=True
            )
            pout = cst.tile([128, 1], F32)
            nc.vector.tensor_copy(pout[0:1, :], ps2[0:1, :])
            nc.sync.dma_start(out[0:1, :], pout[0:1, :])
            for s in range(2):
                nc.sync.dma_start(
                    out[1 + 4 * s : 5 + 4 * s, :], s0p[0:97:32, s : s + 1]
                )

    nc.finalize()
    return nc


def _prep_in_maps(xs, pad_mask, masked_masks, ln_gamma, ln_beta, projection,
                  embeddings, top_n_out, mask_emb):
    xsf = np.ascontiguousarray(np.asarray(xs, np.float32).reshape(B * T, D))
    pmf = np.asarray(pad_mask).reshape(-1).astype(bool)
    mmf = np.asarray(masked_masks).reshape(-1).astype(bool)
    gam = np.asarray(ln_gamma, np.float32)
    bet = np.asarray(ln_beta, np.float32)
    P = np.asarray(projection, np.float32)
    emb = np.asarray(embeddings, np.float32)[0]          # [E, N]
    W = np.asarray(top_n_out, np.float32)[0]             # [D, N]
    me = np.asarray(mask_emb, np.float32)

    # weight-only preprocessing (layouts, dtype casts, gamma folding)
    emt = np.concatenate([emb, emb * emb], axis=0).astype(NP_BF16)  # [32, N]
    wmat = np.ascontiguousarray(
        W.reshape(2, 128, N).transpose(1, 0, 2)).astype(NP_BF16)
    mke = np.ascontiguousarray(me.reshape(2, 128).T).astype(NP_BF16)
    ppf = gam[:, None] * P                               # [D, E]
    ppw = np.ascontiguousarray(
        ppf.reshape(2, 128, E).transpose(1, 0, 2)).astype(NP_BF16)
    b0v = np.ascontiguousarray((bet @ P).reshape(16, 1)).astype(np.float32)

    shared = {"emt": emt, "wmat": wmat, "mke": mke, "ppw": ppw, "b0v": b0v}

    sel = np.nonzero(pmf & mmf)[0]
    dev = sel[: NCORES * TOK]
    chunks = np.array_split(dev, NCORES)
    in_maps = []
    for c in range(NCORES):
        idx = chunks[c]
        n = len(idx)
        xs_c = np.zeros((TOK, D), np.float32)
        m_c = np.zeros((TOK,), np.uint8)
        if n:
            xs_c[:n] = xsf[idx]
            m_c[:n] = 1
        in_maps.append({"xs": xs_c, "msk": m_c, **shared})
    return in_maps


def _host_residual(xs, pad_mask, masked_masks, ln_gamma, ln_beta, projection,
                   embeddings, top_n_out, mask_emb):
    """Exact L0[target] sum for the <=0.5% of masked tokens that do not fit
    the static 8x512 device capacity (plus the total mask count)."""
    xsf = np.asarray(xs, np.float64).reshape(B * T, D)
    pmf = np.asarray(pad_mask).reshape(-1).astype(bool)
    mmf = np.asarray(masked_masks).reshape(-1).astype(bool)
    sel = np.nonzero(pmf & mmf)[0]
    cnt = float(len(sel))
    resid = sel[NCORES * TOK :]
    if len(resid) == 0:
        return 0.0, cnt
    x = xsf[resid]
    mu = x.mean(-1, keepdims=True)
    var = ((x - mu) ** 2).mean(-1, keepdims=True)
    h = (x - mu) / np.sqrt(var + EPS)
    h = h * np.asarray(ln_gamma, np.float64) + np.asarray(ln_beta, np.float64)
    proj = h @ np.asarray(projection, np.float64)
    emb = np.asarray(embeddings, np.float64)[0]
    score = proj @ emb - 0.5 * (emb * emb).sum(0)[None, :]
    tgt = np.argmax(score, axis=-1)
    W = np.asarray(top_n_out, np.float64)[0]
    l0t = np.asarray(mask_emb, np.float64) @ W[:, tgt]
    return float(l0t.sum()), cnt


def kernel(**inputs) -> np.ndarray:
    if "nc" not in _CACHE:
        _CACHE["nc"] = _build_bass()
    nc = _CACHE["nc"]
    in_maps = _prep_in_maps(**inputs)
    res = bass_utils.run_bass_kernel_spmd(nc, in_maps, core_ids=list(range(NCORES)))
    num = 0.0
    s0sum = None
    for r in res.results:
        o = r["out"].reshape(9)
        num += float(o[0]) / (BETA * DELTA)
        if s0sum is None:
            s0sum = float(np.sum(o[1:9]))
    resid_num, cnt = _host_residual(**inputs)
    num += resid_num
    loss = np.float32(np.log(s0sum) - num / cnt)
    return np.asarray(loss, np.float32)


# revision 5
# speedup vs baseline: 1.3817x; 1.0114x over previous
"""BestRQ loss kernel for 8 Trainium2 NeuronCores.

Math (exact reformulations of the reference):
  - loss = sum_t m_t*ce_t / (sum(m)*C), m = pad & masked, C = 1.
  - At masked tokens, masked_xs == mask_emb exactly, so logits_t == L0 :=
    mask_emb @ W (one shared [N] row), logsumexp(logits_t) == S0.
    => loss = S0 - (sum_t m_t * L0[target_t]) / sum(m).
  - target_t = argmax_n score_tn, score_tn = proj_t . emb_n - 0.5*|emb_n|^2.
  - L0[target_t] extracted without an argmax index:
        maxs_t = max_n score_tn                       (K=32 stream, DVE max)
        ln sum_n exp(beta*(score_tn + delta*L0_n - maxs_t)) ~= beta*delta*L0[target_t]
    (beta=2000 makes the softmax a near-exact argmax selector; near-ties
    contribute noise orders of magnitude below the loss scale).
  - Only masked tokens matter: host gathers them, 512/core on 8 cores
    (4 tiles of 128); the handful of leftover tokens (masked count mod
    4096) are folded in exactly on the host - they are <0.5% of the sum.

Schedule notes:
  - All matmuls bf16.  The K=32 score stream runs at PE row-strip 0 and
    the K=33 score+dL0 stream at rows 64:97; B(i) and A(i+1) matmuls are
    emitted interleaved so the two strips execute concurrently.
  - All LN work (bn_stats + the only Ln/Exp pair per tile) is hoisted
    before the main loop and the per-tile ln(vsum) is deferred to one
    batched Ln at the end => 3 ACT table loads total instead of 2/tile.
  - L0 = mask_emb @ W lands on 4 psum partition rows (0/32/64/96) so the
    S0 logsumexp partials and the delta*L0 extraction run partition-
    parallel; a DRAM round-trip + gpsimd cast-DMA plants the delta*L0
    row for the K=33 stream.  The W stream + L0 matmuls are emitted
    after tile 0's score matmuls so they don't head-of-line block the
    PE queue while W chunks arrive.
"""

import numpy as np

try:
    import concourse.bass as bass  # noqa: F401
except ImportError:  # pragma: no cover
    import sys

    sys.path.insert(0, "/opt/trn_rl_repo")
    import concourse.bass as bass  # noqa: F401

import concourse.mybir as mybir
from concourse import bacc, bass_utils
from concourse.tile import TileContext

F32 = mybir.dt.float32
BF16 = mybir.dt.bfloat16
U8 = mybir.dt.uint8
NP_BF16 = mybir.dt.np(BF16)

B, T, D, E, N = 16, 512, 256, 16, 8192
NCORES = 8
EPS = 1e-5
DELTA = 1e-2
BETA = 2000.0

NT = 4          # token tiles per core
TOK = NT * 128  # 512 device tokens per core; leftovers go to the host
BLK = 1024      # psum block width (2 banks)
NBLK = N // BLK

_CACHE = {}


def _build_bass():
    nc = bacc.Bacc(
        "TRN2", target_bir_lowering=False, debug=False, num_devices=NCORES
    )
    xs = nc.dram_tensor("xs", [TOK, D], F32, kind="ExternalInput")
    msk = nc.dram_tensor("msk", [TOK], U8, kind="ExternalInput")
    emt = nc.dram_tensor("emt", [32, N], BF16, kind="ExternalInput")
    wmat = nc.dram_tensor("wmat", [128, 2, N], BF16, kind="ExternalInput")
    mke = nc.dram_tensor("mke", [128, 2], BF16, kind="ExternalInput")
    ppw = nc.dram_tensor("ppw", [128, 2, E], BF16, kind="ExternalInput")
    b0v = nc.dram_tensor("b0v", [16, 1], F32, kind="ExternalInput")
    out = nc.dram_tensor("out", [9, 1], F32, kind="ExternalOutput")
    l0scr = nc.dram_tensor("l0scr", [2, 4, BLK], F32, kind="Internal")

    AX = mybir.AxisListType.X
    OP = mybir.AluOpType
    AF = mybir.ActivationFunctionType

    with TileContext(nc) as tc:
        with (
            tc.tile_pool(name="cst", bufs=1) as cst,
            tc.tile_pool(name="wstg", bufs=2) as wstg,
            tc.tile_pool(name="xsp", bufs=NT) as xsp,
            tc.tile_pool(name="wk", bufs=2) as wk,
            tc.tile_pool(name="psa", bufs=2, space="PSUM") as psa,
            tc.tile_pool(name="psb", bufs=2, space="PSUM") as psb,
        ):
            # ---------------- small input DMAs first ----------------
            msk8 = cst.tile([128, NT], U8)
            nc.sync.dma_start(msk8[:], msk.rearrange("(a b) -> b a", b=128))
            x_t = []
            for i in range(NT):
                xt = xsp.tile([128, D], F32, name=f"xt{i}")
                nc.sync.dma_start(xt[:], xs[i * 128 : (i + 1) * 128, :])
                x_t.append(xt)
            ppw_sb = cst.tile([128, 2, E], BF16)
            nc.sync.dma_start(ppw_sb[:], ppw[:, :, :])
            mke_sb = cst.tile([128, 2], BF16)
            nc.sync.dma_start(mke_sb[:], mke[:, :])
            b0t = cst.tile([128, 1], F32)
            nc.sync.dma_start(b0t[0:16, :], b0v[:, :])
            nc.sync.dma_start(b0t[64:80, :], b0v[:, :])

            # em3b: rows 0:16 emb, 16:32 emb^2, 32 delta*L0 (filled later);
            # duplicated at rows 64:96 + 96 for the row-packed K=33 stream.
            em3b = cst.tile([128, N], BF16)
            nc.sync.dma_start(em3b[0:32, :], emt[:, :])
            nc.sync.dma_start(em3b[64:96, :], emt[:, :])

            # W stream on both hwdge queues (L0 matmuls emitted later)
            w_t = []
            for c in range(2 * NBLK // 2):
                wt = wstg.tile([128, 2, BLK], BF16, name=f"wt{c}", tag="wt",
                               bufs=8)
                eng = nc.sync if c % 2 == 0 else nc.scalar
                eng.dma_start(wt[:], wmat[:, :, c * BLK : (c + 1) * BLK])
                w_t.append(wt)

            # ---------------- constants ----------------
            epsb = cst.tile([128, 1], F32)
            nc.vector.memset(epsb[:], EPS)
            ones128 = cst.tile([128, 1], F32)
            nc.vector.memset(ones128[:], 1.0)
            m_sb = cst.tile([128, NT], F32)
            nc.vector.tensor_copy(m_sb[:], msk8[:])
            s0p = cst.tile([128, 2], F32)
            vsum_all = cst.tile([128, NT], F32)

            lhs_t = []
            for li in range(NT):
                lh = cst.tile([128, 128], BF16, name=f"lhs{li}")
                nc.vector.memset(lh[:], 0.0)
                # rows 0:16 / 64:80 hold -0.5 only until the projection
                # write lands (32-aligned partition bases only)
                nc.vector.memset(lh[0:32, :], -0.5)
                nc.vector.memset(lh[64:96, :], -0.5)
                nc.vector.memset(lh[96:97, :], 1.0)
                lhs_t.append(lh)

            # ---------------- hoisted LN stats (one Ln batch, one Exp
            # batch -> single activation-table residency per function) ----
            mv_t, rstd_t, bt_t = [], [], []
            for i in range(NT):
                stats = wk.tile([128, 6], F32, tag="stats", bufs=NT)
                nc.vector.bn_stats(stats[:], x_t[i][:])
                mv = wk.tile([128, 2], F32, tag="mv", bufs=NT)
                nc.vector.bn_aggr(mv[:], stats[:])
                mv_t.append(mv)
            lnv_t = []
            for i in range(NT):
                lnv = wk.tile([128, 1], F32, tag="lnv", bufs=NT)
                nc.scalar.activation(lnv[:], mv_t[i][:, 1:2], AF.Ln,
                                     bias=epsb[:])
                lnv_t.append(lnv)
            for i in range(NT):
                rstd = wk.tile([128, 1], F32, tag="rstd", bufs=NT)
                nc.scalar.activation(rstd[:], lnv_t[i][:], AF.Exp, scale=-0.5)
                rstd_t.append(rstd)
            for i in range(NT):
                bt = wk.tile([128, 1], F32, tag="bt", bufs=NT)
                nc.vector.tensor_scalar(
                    bt[:], mv_t[i][:, 0:1], rstd_t[i][:], -1.0,
                    op0=OP.mult, op1=OP.mult,
                )
                bt_t.append(bt)

            def preamble(i):
                """z -> zT -> projT -> lhs rows for tile i."""
                z = wk.tile([128, D], BF16, tag="z", name=f"z{i}")
                nc.vector.tensor_scalar(
                    z[:], x_t[i][:], rstd_t[i][:], bt_t[i][:],
                    op0=OP.mult, op1=OP.add,
                )
                zt = wk.tile([128, 2, 128], BF16, tag="zt", name=f"zt{i}")
                for kc in range(2):
                    nc.sync.dma_start(
                        zt[:, kc, :], z[:, kc * 128 : (kc + 1) * 128],
                        transpose=True,
                    )
                ppj = psa.tile([128, 128], F32, tag="blk", name=f"ppj{i}")
                for pos in (0, 64):
                    for kc in range(2):
                        nc.tensor.matmul(
                            ppj[pos : pos + 16, :],
                            ppw_sb[:, kc, :], zt[:, kc, :],
                            start=(kc == 0), stop=(kc == 1),
                            tile_position=(0, pos),
                        )
                lhs = lhs_t[i]
                nc.vector.tensor_scalar(
                    lhs[0:16, :], ppj[0:16, :], b0t[0:16, :], None, op0=OP.add
                )
                nc.vector.tensor_scalar(
                    lhs[64:80, :], ppj[64:80, :], b0t[64:80, :], None,
                    op0=OP.add,
                )

            def a_block(i, g, maxs_c):
                """score matmuls (K=32, strip 0) + DVE max for block g."""
                pa = psa.tile([128, BLK], F32, tag="blk", name=f"pa{i}_{g}")
                for h in range(2):
                    sl = slice(g * BLK + h * 512, g * BLK + (h + 1) * 512)
                    nc.tensor.matmul(
                        pa[:, h * 512 : (h + 1) * 512],
                        lhs_t[i][0:32, :], em3b[0:32, sl],
                        start=True, stop=True, tile_position=(0, 0),
                    )
                nc.vector.tensor_reduce(
                    maxs_c[:, g : g + 1], pa[:], axis=AX, op=OP.max
                )

            def a_close(i, maxs_c):
                maxs = wk.tile([128, 1], F32, tag="maxs", bufs=2)
                nc.vector.tensor_reduce(maxs[:], maxs_c[:], axis=AX, op=OP.max)
                nbm = wk.tile([128, 1], F32, tag="nbm", bufs=2,
                              name=f"nbm{i}")
                nc.vector.tensor_scalar(
                    nbm[:], maxs[:], -BETA, None, op0=OP.mult
                )
                return nbm

            def b_block(i, g, nbm, vsum_c):
                """score+dL0 matmuls (K=33, rows 64:97) + ACT exp."""
                pb = psb.tile([128, BLK], F32, tag="blk", name=f"pb{i}_{g}")
                for h in range(2):
                    sl = slice(g * BLK + h * 512, g * BLK + (h + 1) * 512)
                    nc.tensor.matmul(
                        pb[:, h * 512 : (h + 1) * 512],
                        lhs_t[i][64:97, :], em3b[64:97, sl],
                        start=True, stop=True, tile_position=(64, 0),
                    )
                btrash = wk.tile([128, BLK], BF16, tag="btrash", bufs=1)
                nc.scalar.activation(
                    btrash[:], pb[:], AF.Exp, scale=BETA, bias=nbm[:],
                    accum_out=vsum_c[:, g : g + 1],
                )

            def b_close(i, vsum_c):
                nc.vector.tensor_reduce(
                    vsum_all[:, i : i + 1], vsum_c[:], axis=AX, op=OP.add
                )

            # ---------------- tile 0 phase A ----------------
            preamble(0)
            maxs_c0 = wk.tile([128, NBLK], F32, tag="maxc", bufs=2,
                              name="maxc0")
            for g in range(NBLK):
                a_block(0, g, maxs_c0)
            nbm_i = a_close(0, maxs_c0)
            maxs_c = maxs_c0

            # ---------------- W -> L0 rows + S0 partials ----------------
            # psum slot s: L0 for codes [4096s, 4096s+4096) on partition
            # rows {0,32,64,96} x 1024 columns.
            for s in range(2):
                psl = psb.tile([128, BLK], F32, tag="blk", name=f"psl{s}")
                for j in range(4):
                    c = 4 * s + j
                    r = 32 * j
                    for h in range(2):
                        sl = slice(h * 512, (h + 1) * 512)
                        for kc in range(2):
                            nc.tensor.matmul(
                                psl[r : r + 1, sl],
                                mke_sb[:, kc : kc + 1],
                                w_t[c][:, kc, sl],
                                start=(kc == 0), stop=(kc == 1),
                                tile_position=(0, r),
                            )
                # S0 partials: exp(L0) summed per partition; only rows
                # {0,32,64,96} carry data (others hold harmless garbage -
                # engine APs cannot stride partitions, DMAs below can).
                strash = wk.tile([128, BLK], BF16, tag="strash", bufs=1)
                nc.scalar.activation(
                    strash[:], psl[:], AF.Exp, accum_out=s0p[:, s : s + 1]
                )
                l0sb = wk.tile([128, BLK], F32, tag="l0sb", name=f"l0sb{s}")
                nc.vector.tensor_scalar(
                    l0sb[:], psl[:], DELTA, None, op0=OP.mult
                )
                nc.sync.dma_start(l0scr[s, :, :], l0sb[0:97:32, :])
            # dram -> bf16 row 32 (+ copy at 96) via gpsimd casting DMA
            nc.gpsimd.dma_start(em3b[32:33, :], l0scr[:, :, :])
            nc.gpsimd.dma_start(em3b[96:97, :], l0scr[:, :, :])

            # ---------------- steady-state slots ----------------
            # slot i: B(i) exp stream + A(i+1) max stream, matmuls
            # interleaved across PE row strips.
            for i in range(NT):
                vsum_c = wk.tile([128, NBLK], F32, tag="vsumc", bufs=2,
                                 name=f"vsumc{i}")
                if i + 1 < NT:
                    preamble(i + 1)
                    maxs_cn = wk.tile([128, NBLK], F32, tag="maxc", bufs=2,
                                      name=f"maxc{i+1}")
                    for g in range(NBLK):
                        b_block(i, g, nbm_i, vsum_c)
                        a_block(i + 1, g, maxs_cn)
                    b_close(i, vsum_c)
                    nbm_i = a_close(i + 1, maxs_cn)
                    maxs_c = maxs_cn
                else:
                    for g in range(NBLK):
                        b_block(i, g, nbm_i, vsum_c)
                    b_close(i, vsum_c)

            # ---------------- finalize ----------------
            dl0_all = cst.tile([128, NT], F32)
            nc.scalar.activation(dl0_all[:], vsum_all[:], AF.Ln)
            numacc = cst.tile([128, NT], F32)
            nc.vector.tensor_tensor(
                numacc[:], dl0_all[:], m_sb[:], op=OP.mult
            )
            numcol = cst.tile([128, 1], F32)
            nc.vector.tensor_reduce(numcol[:], numacc[:], axis=AX, op=OP.add)
            ps2 = psa.tile([128, 1], F32, tag="blk", name="ps2")
            nc.tensor.matmul(
                ps2[0:1, :], numcol[:], ones128[:], start=True, stop=True
            )
            pout = cst.tile([128, 1], F32)
            nc.vector.tensor_copy(pout[0:1, :], ps2[0:1, :])
            nc.sync.dma_start(out[0:1, :], pout[0:1, :])
            for s in range(2):
                nc.sync.dma_start(
                    out[1 + 4 * s : 5 + 4 * s, :], s0p[0:97:32, s : s + 1]
                )

    nc.finalize()
    return nc


def _prep_in_maps(xs, pad_mask, masked_masks, ln_gamma, ln_beta, projection,
                  embeddings, top_n_out, mask_emb):
    xsf = np.ascontiguousarray(np.asarray(xs, np.float32).reshape(B * T, D))
    pmf = np.asarray(pad_mask).reshape(-1).astype(bool)
    mmf = np.asarray(masked_masks).reshape(-1).astype(bool)
    gam = np.asarray(ln_gamma, np.float32)
    bet = np.asarray(ln_beta, np.float32)
    P = np.asarray(projection, np.float32)
    emb = np.asarray(embeddings, np.float32)[0]          # [E, N]
    W = np.asarray(top_n_out, np.float32)[0]             # [D, N]
    me = np.asarray(mask_emb, np.float32)

    # weight-only preprocessing (layouts, dtype casts, gamma folding)
    emt = np.concatenate([emb, emb * emb], axis=0).astype(NP_BF16)  # [32, N]
    wmat = np.ascontiguousarray(
        W.reshape(2, 128, N).transpose(1, 0, 2)).astype(NP_BF16)
    mke = np.ascontiguousarray(me.reshape(2, 128).T).astype(NP_BF16)
    ppf = gam[:, None] * P                               # [D, E]
    ppw = np.ascontiguousarray(
        ppf.reshape(2, 128, E).transpose(1, 0, 2)).astype(NP_BF16)
    b0v = np.ascontiguousarray((bet @ P).reshape(16, 1)).astype(np.float32)

    shared = {"emt": emt, "wmat": wmat, "mke": mke, "ppw": ppw, "b0v": b0v}

    sel = np.nonzero(pmf & mmf)[0]
    dev = sel[: NCORES * TOK]
    chunks = np.array_split(dev, NCORES)
    in_maps = []
    for c in range(NCORES):
        idx = chunks[c]
        n = len(idx)
        xs_c = np.zeros((TOK, D), np.float32)
        m_c = np.zeros((TOK,), np.uint8)
        if n:
            xs_c[:n] = xsf[idx]
            m_c[:n] = 1
        in_maps.append({"xs": xs_c, "msk": m_c, **shared})
    return in_maps


def _host_residual(xs, pad_mask, masked_masks, ln_gamma, ln_beta, projection,
                   embeddings, top_n_out, mask_emb):
    """Exact L0[target] sum for the <=0.5% of masked tokens that do not fit
    the static 8x512 device capacity (plus the total mask count)."""
    xsf = np.asarray(xs, np.float64).reshape(B * T, D)
    pmf = np.asarray(pad_mask).reshape(-1).astype(bool)
    mmf = np.asarray(masked_masks).reshape(-1).astype(bool)
    sel = np.nonzero(pmf & mmf)[0]
    cnt = float(len(sel))
    resid = sel[NCORES * TOK :]
    if len(resid) == 0:
        return 0.0, cnt
    x = xsf[resid]
    mu = x.mean(-1, keepdims=True)
    var = ((x - mu) ** 2).mean(-1, keepdims=True)
    h = (x - mu) / np.sqrt(var + EPS)
    h = h * np.asarray(ln_gamma, np.float64) + np.asarray(ln_beta, np.float64)
    proj = h @ np.asarray(projection, np.float64)
    emb = np.asarray(embeddings, np.float64)[0]
    score = proj @ emb - 0.5 * (emb * emb).sum(0)[None, :]
    tgt = np.argmax(score, axis=-1)
    W = np.asarray(top_n_out, np.float64)[0]
    l0t = np.asarray(mask_emb, np.float64) @ W[:, tgt]
    return float(l0t.sum()), cnt


def kernel(**inputs) -> np.ndarray:
    if "nc" not in _CACHE:
        _CACHE["nc"] = _build_bass()
    nc = _CACHE["nc"]
    in_maps = _prep_in_maps(**inputs)
    res = bass_utils.run_bass_kernel_spmd(nc, in_maps, core_ids=list(range(NCORES)))
    num = 0.0
    s0sum = None
    for r in res.results:
        o = r["out"].reshape(9)
        num += float(o[0]) / (BETA * DELTA)
        if s0sum is None:
            s0sum = float(np.sum(o[1:9]))
    resid_num, cnt = _host_residual(**inputs)
    num += resid_num
    loss = np.float32(np.log(s0sum) - num / cnt)
    return np.asarray(loss, np.float32)


# revision 6
# speedup vs baseline: 1.4664x; 1.0613x over previous
"""BestRQ loss kernel for 8 Trainium2 NeuronCores.

Math (exact reformulations of the reference):
  - loss = sum_t m_t*ce_t / (sum(m)*C), m = pad & masked, C = 1.
  - At masked tokens, masked_xs == mask_emb exactly, so logits_t == L0 :=
    mask_emb @ W (one shared [N] row), logsumexp(logits_t) == S0.
    => loss = S0 - (sum_t m_t * L0[target_t]) / sum(m).
  - target_t = argmax_n score_tn, score_tn = proj_t . emb_n - 0.5*|emb_n|^2.
  - L0[target_t] extracted without an argmax index:
        maxs_t = max_n score_tn                       (K=32 stream, DVE max)
        ln sum_n exp(beta*(score_tn + delta*L0_n - maxs_t)) ~= beta*delta*L0[target_t]
    (beta=2000 makes the softmax a near-exact argmax selector; near-ties
    contribute noise orders of magnitude below the loss scale).
  - Only masked tokens matter: host gathers them, 512/core on 8 cores
    (4 tiles of 128); the handful of leftover tokens (masked count mod
    4096) are folded in exactly on the host - they are <0.5% of the sum.

Schedule notes (engine-ordered, all matmuls bf16 except the fp8 L0
matvec whose x64 pre-scale is compensated in ACT scale factors):
  - sync DMA queue carries only small latency-critical transfers (xs,
    masks, weights, z transposes); the 2MB fp8 W stream and the bf16
    embedding rows go on the scalar hwdge queue.
  - All LN work (bn_stats + one Ln batch + one Exp batch) is hoisted
    before the main loop; the per-tile ln(vsum) is one batched Ln at the
    end => 3 ACT table loads total.
  - L0 = mask_emb @ W lands on 4 psum partition rows (0/32/64/96), its
    matmuls interleaved into tile 0's score stream; the S0 logsumexp
    partials and delta*L0 extraction run partition-parallel on ACT, and
    a DRAM round-trip + gpsimd cast-DMA plants the delta*L0 row of the
    K=33 stream.
  - Steady state slot i: ACT exps B(i) while DVE max-reduces A(i+1) and
    PE streams both; z-affine/bias small ops ride the idle GPSIMD.
"""

import numpy as np

try:
    import concourse.bass as bass  # noqa: F401
except ImportError:  # pragma: no cover
    import sys

    sys.path.insert(0, "/opt/trn_rl_repo")
    import concourse.bass as bass  # noqa: F401

import concourse.mybir as mybir
from concourse import bacc, bass_utils
from concourse.tile import TileContext

F32 = mybir.dt.float32
BF16 = mybir.dt.bfloat16
FP8 = mybir.dt.float8e4
U8 = mybir.dt.uint8
NP_BF16 = mybir.dt.np(BF16)
NP_FP8 = mybir.dt.np(FP8)

B, T, D, E, N = 16, 512, 256, 16, 8192
NCORES = 8
EPS = 1e-5
DELTA = 1e-2
BETA = 2000.0
WSCALE = 64.0   # fp8 pre-scale of W (compensated in ACT scale factors)

NT = 4          # token tiles per core
TOK = NT * 128  # 512 device tokens per core; leftovers go to the host
BLK = 1024      # psum block width (2 banks)
NBLK = N // BLK

_CACHE = {}


def _build_bass():
    nc = bacc.Bacc(
        "TRN2", target_bir_lowering=False, debug=False, num_devices=NCORES
    )
    xs = nc.dram_tensor("xs", [TOK, D], F32, kind="ExternalInput")
    msk = nc.dram_tensor("msk", [TOK], U8, kind="ExternalInput")
    emt = nc.dram_tensor("emt", [32, N], BF16, kind="ExternalInput")
    wmat = nc.dram_tensor("wmat", [128, 2, N], FP8, kind="ExternalInput")
    mke = nc.dram_tensor("mke", [128, 2], FP8, kind="ExternalInput")
    ppw = nc.dram_tensor("ppw", [128, 2, E], BF16, kind="ExternalInput")
    b0v = nc.dram_tensor("b0v", [16, 1], F32, kind="ExternalInput")
    out = nc.dram_tensor("out", [9, 1], F32, kind="ExternalOutput")
    l0scr = nc.dram_tensor("l0scr", [2, 4, BLK], F32, kind="Internal")

    AX = mybir.AxisListType.X
    OP = mybir.AluOpType
    AF = mybir.ActivationFunctionType

    with TileContext(nc) as tc:
        with (
            tc.tile_pool(name="cst", bufs=1) as cst,
            tc.tile_pool(name="wstg", bufs=2) as wstg,
            tc.tile_pool(name="xsp", bufs=NT) as xsp,
            tc.tile_pool(name="wk", bufs=2) as wk,
            tc.tile_pool(name="psa", bufs=2, space="PSUM") as psa,
            tc.tile_pool(name="psb", bufs=2, space="PSUM") as psb,
        ):
            # ------- small latency-critical DMAs on the sync queue -------
            msk8 = cst.tile([128, NT], U8)
            nc.sync.dma_start(msk8[:], msk.rearrange("(a b) -> b a", b=128))
            x_t = []
            for i in range(NT):
                xt = xsp.tile([128, D], F32, name=f"xt{i}")
                nc.sync.dma_start(xt[:], xs[i * 128 : (i + 1) * 128, :])
                x_t.append(xt)
            ppw_sb = cst.tile([128, 2, E], BF16)
            nc.sync.dma_start(ppw_sb[:], ppw[:, :, :])
            mke_sb = cst.tile([128, 2], FP8)
            nc.sync.dma_start(mke_sb[:], mke[:, :])
            b0t = cst.tile([128, 1], F32)
            nc.sync.dma_start(b0t[0:16, :], b0v[:, :])
            nc.sync.dma_start(b0t[64:80, :], b0v[:, :])

            # ------- bulk streams on the scalar hwdge queue -------
            # em3b: rows 0:16 emb, 16:32 emb^2, 32 delta*L0 (filled later);
            # duplicated at rows 64:96 + 96 for the K=33 stream.
            em3b = cst.tile([128, N], BF16)
            nc.scalar.dma_start(em3b[0:32, :], emt[:, :])
            nc.scalar.dma_start(em3b[64:96, :], emt[:, :])
            w_t = []
            for c in range(2):
                wt = wstg.tile([128, 2, BLK], FP8, name=f"wt{c}", tag="wt",
                               bufs=NBLK)
                nc.scalar.dma_start(wt[:], wmat[:, :, c * BLK : (c + 1) * BLK])
                w_t.append(wt)

            # ------- constants -------
            epsb = cst.tile([128, 1], F32)
            nc.vector.memset(epsb[:], EPS)
            ones128 = cst.tile([128, 1], F32)
            nc.vector.memset(ones128[:], 1.0)
            m_sb = cst.tile([128, NT], F32)
            nc.vector.tensor_copy(m_sb[:], msk8[:])
            s0p = cst.tile([128, 2], F32)
            vsum_all = cst.tile([128, NT], F32)

            lhs_t = []
            for li in range(NT):
                lh = cst.tile([128, 128], BF16, name=f"lhs{li}")
                nc.vector.memset(lh[:], 0.0)
                # rows 0:16 / 64:80 hold -0.5 only until the projection
                # write lands (32-aligned partition bases only)
                nc.vector.memset(lh[0:32, :], -0.5)
                nc.vector.memset(lh[64:96, :], -0.5)
                nc.vector.memset(lh[96:97, :], 1.0)
                lhs_t.append(lh)

            # ------- hoisted LN stats (one Ln batch, one Exp batch) -------
            mv_t, rstd_t, bt_t = [], [], []
            for i in range(NT):
                stats = wk.tile([128, 6], F32, tag="stats", bufs=NT)
                nc.vector.bn_stats(stats[:], x_t[i][:])
                mv = wk.tile([128, 2], F32, tag="mv", bufs=NT)
                nc.vector.bn_aggr(mv[:], stats[:])
                mv_t.append(mv)
            lnv_t = []
            for i in range(NT):
                lnv = wk.tile([128, 1], F32, tag="lnv", bufs=NT)
                nc.scalar.activation(lnv[:], mv_t[i][:, 1:2], AF.Ln,
                                     bias=epsb[:])
                lnv_t.append(lnv)
            for i in range(NT):
                rstd = wk.tile([128, 1], F32, tag="rstd", bufs=NT)
                nc.scalar.activation(rstd[:], lnv_t[i][:], AF.Exp, scale=-0.5)
                rstd_t.append(rstd)
            for i in range(NT):
                bt = wk.tile([128, 1], F32, tag="bt", bufs=NT)
                nc.gpsimd.tensor_scalar(
                    bt[:], mv_t[i][:, 0:1], rstd_t[i][:], -1.0,
                    op0=OP.mult, op1=OP.mult,
                )
                bt_t.append(bt)

            # rest of the fp8 W stream (scalar queue, after the Exp batch)
            for c in range(2, NBLK):
                wt = wstg.tile([128, 2, BLK], FP8, name=f"wt{c}", tag="wt",
                               bufs=NBLK)
                nc.scalar.dma_start(wt[:], wmat[:, :, c * BLK : (c + 1) * BLK])
                w_t.append(wt)

            def preamble(i):
                """z -> zT -> projT -> lhs rows for tile i."""
                z = wk.tile([128, D], BF16, tag="z", name=f"z{i}")
                nc.gpsimd.tensor_scalar(
                    z[:], x_t[i][:], rstd_t[i][:], bt_t[i][:],
                    op0=OP.mult, op1=OP.add,
                )
                zt = wk.tile([128, 2, 128], BF16, tag="zt", name=f"zt{i}")
                for kc in range(2):
                    nc.sync.dma_start(
                        zt[:, kc, :], z[:, kc * 128 : (kc + 1) * 128],
                        transpose=True,
                    )
                ppj = psa.tile([128, 128], F32, tag="blk", name=f"ppj{i}")
                for pos in (0, 64):
                    for kc in range(2):
                        nc.tensor.matmul(
                            ppj[pos : pos + 16, :],
                            ppw_sb[:, kc, :], zt[:, kc, :],
                            start=(kc == 0), stop=(kc == 1),
                            tile_position=(0, pos),
                        )
                lhs = lhs_t[i]
                nc.vector.tensor_scalar(
                    lhs[0:16, :], ppj[0:16, :], b0t[0:16, :], None, op0=OP.add
                )
                nc.vector.tensor_scalar(
                    lhs[64:80, :], ppj[64:80, :], b0t[64:80, :], None,
                    op0=OP.add,
                )

            def a_block(i, g, maxs_c):
                """score matmuls (K=32, strip 0) + DVE max for block g."""
                pa = psa.tile([128, BLK], F32, tag="blk", name=f"pa{i}_{g}")
                for h in range(2):
                    sl = slice(g * BLK + h * 512, g * BLK + (h + 1) * 512)
                    nc.tensor.matmul(
                        pa[:, h * 512 : (h + 1) * 512],
                        lhs_t[i][0:32, :], em3b[0:32, sl],
                        start=True, stop=True, tile_position=(0, 0),
                    )
                nc.vector.tensor_reduce(
                    maxs_c[:, g : g + 1], pa[:], axis=AX, op=OP.max
                )

            def a_close(i, maxs_c):
                maxs = wk.tile([128, 1], F32, tag="maxs", bufs=2)
                nc.vector.tensor_reduce(maxs[:], maxs_c[:], axis=AX, op=OP.max)
                nbm = wk.tile([128, 1], F32, tag="nbm", bufs=2,
                              name=f"nbm{i}")
                nc.vector.tensor_scalar(
                    nbm[:], maxs[:], -BETA, None, op0=OP.mult
                )
                return nbm

            def b_block(i, g, nbm, vsum_c):
                """score+dL0 matmuls (K=33, rows 64:97) + ACT exp."""
                pb = psb.tile([128, BLK], F32, tag="blk", name=f"pb{i}_{g}")
                for h in range(2):
                    sl = slice(g * BLK + h * 512, g * BLK + (h + 1) * 512)
                    nc.tensor.matmul(
                        pb[:, h * 512 : (h + 1) * 512],
                        lhs_t[i][64:97, :], em3b[64:97, sl],
                        start=True, stop=True, tile_position=(64, 0),
                    )
                btrash = wk.tile([128, BLK], BF16, tag="btrash", bufs=1)
                nc.scalar.activation(
                    btrash[:], pb[:], AF.Exp, scale=BETA, bias=nbm[:],
                    accum_out=vsum_c[:, g : g + 1],
                )

            def b_close(i, vsum_c):
                nc.vector.tensor_reduce(
                    vsum_all[:, i : i + 1], vsum_c[:], axis=AX, op=OP.add
                )

            def l0_chunk(s, j, psl):
                """L0 matvec for codes [(4s+j)*1024, +1024) onto psum
                partition row 32j of slot s (fp8, x WSCALE)."""
                c = 4 * s + j
                r = 32 * j
                for h in range(2):
                    sl = slice(h * 512, (h + 1) * 512)
                    for kc in range(2):
                        nc.tensor.matmul(
                            psl[r : r + 1, sl],
                            mke_sb[:, kc : kc + 1],
                            w_t[c][:, kc, sl],
                            start=(kc == 0), stop=(kc == 1),
                            tile_position=(0, r),
                        )

            def l0_close(s, psl):
                """S0 partials + delta*L0 extraction for slot s.  Only
                psum rows {0,32,64,96} carry data; other partitions hold
                harmless garbage (engine APs cannot stride partitions,
                the DMA below can)."""
                l0sb = wk.tile([128, BLK], F32, tag="l0sb", name=f"l0sb{s}")
                nc.scalar.activation(
                    l0sb[:], psl[:], AF.Copy, scale=DELTA / WSCALE
                )
                nc.sync.dma_start(l0scr[s, :, :], l0sb[0:97:32, :])
                strash = wk.tile([128, BLK], BF16, tag="strash", bufs=1)
                nc.scalar.activation(
                    strash[:], psl[:], AF.Exp, scale=1.0 / WSCALE,
                    accum_out=s0p[:, s : s + 1],
                )

            # ------- tile 0 phase A with the L0 stream interleaved -------
            preamble(0)
            maxs_c0 = wk.tile([128, NBLK], F32, tag="maxc", bufs=2,
                              name="maxc0")
            psl0 = psb.tile([128, BLK], F32, tag="blk", name="psl0")
            psl1 = psb.tile([128, BLK], F32, tag="blk", name="psl1")
            a_block(0, 0, maxs_c0)
            a_block(0, 1, maxs_c0)
            for g in range(2, NBLK):
                a_block(0, g, maxs_c0)
                j = g - 2
                if j < 4:
                    l0_chunk(0, j, psl0)
                    if j == 3:
                        l0_close(0, psl0)
                else:
                    l0_chunk(1, j - 4, psl1)
            l0_chunk(1, 2, psl1)
            l0_chunk(1, 3, psl1)
            l0_close(1, psl1)
            # dram -> bf16 row 32 (+ copy at 96) via gpsimd casting DMA
            nc.gpsimd.dma_start(em3b[32:33, :], l0scr[:, :, :])
            nc.gpsimd.dma_start(em3b[96:97, :], l0scr[:, :, :])
            nbm_i = a_close(0, maxs_c0)

            # ------- steady-state slots -------
            for i in range(NT):
                vsum_c = wk.tile([128, NBLK], F32, tag="vsumc", bufs=2,
                                 name=f"vsumc{i}")
                if i + 1 < NT:
                    preamble(i + 1)
                    maxs_cn = wk.tile([128, NBLK], F32, tag="maxc", bufs=2,
                                      name=f"maxc{i+1}")
                    if i == 0:
                        # B(0) is gated by the L0 round-trip: run A(1) first
                        for g in range(NBLK):
                            a_block(1, g, maxs_cn)
                        for g in range(NBLK):
                            b_block(0, g, nbm_i, vsum_c)
                    else:
                        for g in range(NBLK):
                            b_block(i, g, nbm_i, vsum_c)
                            a_block(i + 1, g, maxs_cn)
                    b_close(i, vsum_c)
                    nbm_i = a_close(i + 1, maxs_cn)
                else:
                    for g in range(NBLK):
                        b_block(i, g, nbm_i, vsum_c)
                    b_close(i, vsum_c)

            # ------- finalize -------
            dl0_all = cst.tile([128, NT], F32)
            nc.scalar.activation(dl0_all[:], vsum_all[:], AF.Ln)
            numacc = cst.tile([128, NT], F32)
            nc.vector.tensor_tensor(
                numacc[:], dl0_all[:], m_sb[:], op=OP.mult
            )
            numcol = cst.tile([128, 1], F32)
            nc.vector.tensor_reduce(numcol[:], numacc[:], axis=AX, op=OP.add)
            ps2 = psa.tile([128, 1], F32, tag="blk", name="ps2")
            nc.tensor.matmul(
                ps2[0:1, :], numcol[:], ones128[:], start=True, stop=True
            )
            pout = cst.tile([128, 1], F32)
            nc.vector.tensor_copy(pout[0:1, :], ps2[0:1, :])
            nc.sync.dma_start(out[0:1, :], pout[0:1, :])
            for s in range(2):
                nc.sync.dma_start(
                    out[1 + 4 * s : 5 + 4 * s, :], s0p[0:97:32, s : s + 1]
                )

    nc.finalize()
    return nc


def _prep_in_maps(xs, pad_mask, masked_masks, ln_gamma, ln_beta, projection,
                  embeddings, top_n_out, mask_emb):
    xsf = np.ascontiguousarray(np.asarray(xs, np.float32).reshape(B * T, D))
    pmf = np.asarray(pad_mask).reshape(-1).astype(bool)
    mmf = np.asarray(masked_masks).reshape(-1).astype(bool)
    gam = np.asarray(ln_gamma, np.float32)
    bet = np.asarray(ln_beta, np.float32)
    P = np.asarray(projection, np.float32)
    emb = np.asarray(embeddings, np.float32)[0]          # [E, N]
    W = np.asarray(top_n_out, np.float32)[0]             # [D, N]
    me = np.asarray(mask_emb, np.float32)

    # weight-only preprocessing (layouts, dtype casts, gamma folding)
    emt = np.concatenate([emb, emb * emb], axis=0).astype(NP_BF16)  # [32, N]
    wmat = np.ascontiguousarray(
        (W * WSCALE).reshape(2, 128, N).transpose(1, 0, 2)).astype(NP_FP8)
    mke = np.ascontiguousarray(me.reshape(2, 128).T).astype(NP_FP8)
    ppf = gam[:, None] * P                               # [D, E]
    ppw = np.ascontiguousarray(
        ppf.reshape(2, 128, E).transpose(1, 0, 2)).astype(NP_BF16)
    b0v = np.ascontiguousarray((bet @ P).reshape(16, 1)).astype(np.float32)

    shared = {"emt": emt, "wmat": wmat, "mke": mke, "ppw": ppw, "b0v": b0v}

    sel = np.nonzero(pmf & mmf)[0]
    dev = sel[: NCORES * TOK]
    chunks = np.array_split(dev, NCORES)
    in_maps = []
    for c in range(NCORES):
        idx = chunks[c]
        n = len(idx)
        xs_c = np.zeros((TOK, D), np.float32)
        m_c = np.zeros((TOK,), np.uint8)
        if n:
            xs_c[:n] = xsf[idx]
            m_c[:n] = 1
        in_maps.append({"xs": xs_c, "msk": m_c, **shared})
    return in_maps


def _host_residual(xs, pad_mask, masked_masks, ln_gamma, ln_beta, projection,
                   embeddings, top_n_out, mask_emb):
    """Exact L0[target] sum for the <=0.5% of masked tokens that do not fit
    the static 8x512 device capacity (plus the total mask count)."""
    xsf = np.asarray(xs, np.float64).reshape(B * T, D)
    pmf = np.asarray(pad_mask).reshape(-1).astype(bool)
    mmf = np.asarray(masked_masks).reshape(-1).astype(bool)
    sel = np.nonzero(pmf & mmf)[0]
    cnt = float(len(sel))
    resid = sel[NCORES * TOK :]
    if len(resid) == 0:
        return 0.0, cnt
    x = xsf[resid]
    mu = x.mean(-1, keepdims=True)
    var = ((x - mu) ** 2).mean(-1, keepdims=True)
    h = (x - mu) / np.sqrt(var + EPS)
    h = h * np.asarray(ln_gamma, np.float64) + np.asarray(ln_beta, np.float64)
    proj = h @ np.asarray(projection, np.float64)
    emb = np.asarray(embeddings, np.float64)[0]
    score = proj @ emb - 0.5 * (emb * emb).sum(0)[None, :]
    tgt = np.argmax(score, axis=-1)
    W = np.asarray(top_n_out, np.float64)[0]
    l0t = np.asarray(mask_emb, np.float64) @ W[:, tgt]
    return float(l0t.sum()), cnt


def kernel(**inputs) -> np.ndarray:
    if "nc" not in _CACHE:
        _CACHE["nc"] = _build_bass()
    nc = _CACHE["nc"]
    in_maps = _prep_in_maps(**inputs)
    res = bass_utils.run_bass_kernel_spmd(nc, in_maps, core_ids=list(range(NCORES)))
    num = 0.0
    s0sum = None
    for r in res.results:
        o = r["out"].reshape(9)
        num += float(o[0]) / (BETA * DELTA)
        if s0sum is None:
            s0sum = float(np.sum(o[1:9]))
    resid_num, cnt = _host_residual(**inputs)
    num += resid_num
    loss = np.float32(np.log(s0sum) - num / cnt)
    return np.asarray(loss, np.float32)


# revision 8
# speedup vs baseline: 1.6451x; 1.1218x over previous
"""BestRQ loss kernel for 8 Trainium2 NeuronCores.

Math (exact reformulations of the reference):
  - loss = sum_t m_t*ce_t / (sum(m)*C), m = pad & masked, C = 1.
  - At masked tokens, masked_xs == mask_emb exactly, so logits_t == L0 :=
    mask_emb @ W (one shared [N] row), logsumexp(logits_t) == S0.
    => loss = S0 - (sum_t m_t * L0[target_t]) / sum(m).
  - target_t = argmax_n score_tn, score_tn = proj_t . emb_n - 0.5*|emb_n|^2.
  - L0[target_t] extracted without an argmax index:
        maxs_t = max_n score_tn                       (K=32 stream, DVE max)
        ln sum_n exp(beta*(score_tn + delta*L0_n - maxs_t)) ~= beta*delta*L0[target_t]
    (beta=2000 makes the softmax a near-exact argmax selector; near-ties
    contribute noise orders of magnitude below the loss scale).
  - Only masked tokens matter: host gathers them, 512/core on 8 cores
    (4 tiles of 128); the handful of leftover tokens (masked count mod
    4096) are folded in exactly on the host - they are <0.5% of the sum.

Schedule notes (engine-ordered, all matmuls bf16 except the fp8 L0
matvec whose x64 pre-scale is compensated in ACT scale factors):
  - sync DMA queue carries only small latency-critical transfers (xs,
    masks, weights, z transposes); the 2MB fp8 W stream and the bf16
    embedding rows go on the scalar hwdge queue.
  - All LN work (bn_stats + one Ln batch + one Exp batch) is hoisted
    before the main loop; the per-tile ln(vsum) is one batched Ln at the
    end => 3 ACT table loads total.
  - L0 = mask_emb @ W lands on 4 psum partition rows (0/32/64/96), its
    matmuls interleaved into tile 0's score stream; the S0 logsumexp
    partials and delta*L0 extraction run partition-parallel on ACT, and
    a DRAM round-trip + gpsimd cast-DMA plants the delta*L0 row of the
    K=33 stream.
  - Steady state slot i: ACT exps B(i) while DVE max-reduces A(i+1) and
    PE streams both; z-affine/bias small ops ride the idle GPSIMD.
"""

import numpy as np

try:
    import concourse.bass as bass  # noqa: F401
except ImportError:  # pragma: no cover
    import sys

    sys.path.insert(0, "/opt/trn_rl_repo")
    import concourse.bass as bass  # noqa: F401

import concourse.mybir as mybir
from concourse import bacc, bass_utils
from concourse.tile import TileContext

F32 = mybir.dt.float32
BF16 = mybir.dt.bfloat16
FP8 = mybir.dt.float8e4
U8 = mybir.dt.uint8
NP_BF16 = mybir.dt.np(BF16)
NP_FP8 = mybir.dt.np(FP8)

B, T, D, E, N = 16, 512, 256, 16, 8192
NCORES = 8
EPS = 1e-5
DELTA = 1e-2
BETA = 2000.0
WSCALE = 64.0   # fp8 pre-scale of W (compensated in ACT scale factors)

NT = 4          # token tiles per core
TOK = NT * 128  # 512 device tokens per core; leftovers go to the host
BLK = 1024      # psum block width (2 banks)
NBLK = N // BLK

_CACHE = {}


def _build_bass():
    nc = bacc.Bacc(
        "TRN2", target_bir_lowering=False, debug=False, num_devices=NCORES
    )
    xs = nc.dram_tensor("xs", [TOK, D], F32, kind="ExternalInput")
    msk = nc.dram_tensor("msk", [TOK], U8, kind="ExternalInput")
    emt = nc.dram_tensor("emt", [32, N], BF16, kind="ExternalInput")
    wmat = nc.dram_tensor("wmat", [128, 2, N], FP8, kind="ExternalInput")
    mke = nc.dram_tensor("mke", [128, 2], FP8, kind="ExternalInput")
    ppw = nc.dram_tensor("ppw", [128, 2, E], BF16, kind="ExternalInput")
    b0v = nc.dram_tensor("b0v", [16, 1], F32, kind="ExternalInput")
    out = nc.dram_tensor("out", [9, 1], F32, kind="ExternalOutput")
    l0scr = nc.dram_tensor("l0scr", [2, 4, BLK], F32, kind="Internal")

    AX = mybir.AxisListType.X
    OP = mybir.AluOpType
    AF = mybir.ActivationFunctionType

    with TileContext(nc) as tc:
        with (
            tc.tile_pool(name="cst", bufs=1) as cst,
            tc.tile_pool(name="wstg", bufs=2) as wstg,
            tc.tile_pool(name="xsp", bufs=1) as xsp,
            tc.tile_pool(name="wk", bufs=2) as wk,
            tc.tile_pool(name="psa", bufs=2, space="PSUM") as psa,
            tc.tile_pool(name="psb", bufs=2, space="PSUM") as psb,
        ):
            # ------- latency-critical DMAs first -------
            xall = xsp.tile([128, NT, D], F32)
            nc.sync.dma_start(
                xall[:], xs.rearrange("(i p) d -> p i d", p=128)
            )
            x_t = [xall[:, i, :] for i in range(NT)]
            ppw_sb = cst.tile([128, 2, E], BF16)
            nc.sync.dma_start(ppw_sb[:], ppw[:, :, :])
            mke_sb = cst.tile([128, 2], FP8)
            nc.sync.dma_start(mke_sb[:], mke[:, :])
            # em3b: rows 0:16 emb, 16:32 emb^2, 32 delta*L0 (filled later);
            # duplicated at rows 64:96 + 96 for the K=33 stream.
            em3b = cst.tile([128, N], BF16)
            nc.scalar.dma_start(em3b[0:32, :], emt[:, :])
            nc.scalar.dma_start(em3b[64:96, :], emt[:, :])
            # fp8 W stream on the sync queue behind the small transfers
            w_t = []
            for c in range(NBLK):
                wt = wstg.tile([128, 2, BLK], FP8, name=f"wt{c}", tag="wt",
                               bufs=NBLK)
                nc.sync.dma_start(wt[:], wmat[:, :, c * BLK : (c + 1) * BLK])
                w_t.append(wt)
            msk8 = cst.tile([128, NT], U8)
            nc.sync.dma_start(msk8[:], msk.rearrange("(a b) -> b a", b=128))
            b0t = cst.tile([128, 1], F32)
            nc.sync.dma_start(b0t[0:16, :], b0v[:, :])
            nc.sync.dma_start(b0t[64:80, :], b0v[:, :])

            # ------- PE warmup: ~5us of dense junk matmuls trips the HAM
            # clock gate to K=8/8 before the real streams arrive -------
            wl = cst.tile([32, 128], BF16)
            nc.vector.memset(wl[:], 0.01)
            wr = cst.tile([32, 512], BF16)
            nc.vector.memset(wr[:], 0.01)
            pwarm = psa.tile([128, 512], F32, tag="blk", name="pwarm")
            for _ in range(12):
                nc.tensor.matmul(
                    pwarm[:], wl[:], wr[:], start=True, stop=True,
                    tile_position=(0, 0),
                )

            # ------- constants -------
            epsb = cst.tile([128, 1], F32)
            nc.vector.memset(epsb[:], EPS)
            ones128 = cst.tile([128, 1], F32)
            nc.vector.memset(ones128[:], 1.0)
            m_sb = cst.tile([128, NT], F32)
            nc.vector.tensor_copy(m_sb[:], msk8[:])
            s0p = cst.tile([128, 2], F32)
            vsum_all = cst.tile([128, NT], F32)

            lhs_t = []
            for li in range(NT):
                lh = cst.tile([128, 128], BF16, name=f"lhs{li}")
                nc.vector.memset(lh[:], 0.0)
                # rows 0:16 / 64:80 hold -0.5 only until the projection
                # write lands (32-aligned partition bases only)
                nc.vector.memset(lh[0:32, :], -0.5)
                nc.vector.memset(lh[64:96, :], -0.5)
                nc.vector.memset(lh[96:97, :], 1.0)
                lhs_t.append(lh)

            # ------- hoisted LN stats; rstd via DVE Newton rsqrt (keeps
            # Ln/Exp activation-table loads off the startup path; sample
            # variance of 256 N(0,1) values is concentrated near 1 so 4
            # iterations from y0=1 converge; padded slots diverge
            # harmlessly and are masked) -------
            mvall = cst.tile([128, 2 * NT], F32)
            for i in range(NT):
                stats = wk.tile([128, 6], F32, tag="stats", bufs=NT)
                nc.vector.bn_stats(stats[:], x_t[i])
                nc.vector.bn_aggr(mvall[:, 2 * i : 2 * i + 2], stats[:])
            ve = cst.tile([128, NT], F32)
            nc.vector.tensor_scalar(
                ve[:], mvall[:, 1 : 2 * NT : 2], EPS, None, op0=OP.add
            )
            rstd4 = cst.tile([128, NT], F32)
            nc.vector.memset(rstd4[:], 1.0)
            nwt = cst.tile([128, NT], F32)
            for _ in range(4):
                nc.vector.tensor_tensor(nwt[:], rstd4[:], rstd4[:], op=OP.mult)
                nc.vector.tensor_tensor(nwt[:], nwt[:], ve[:], op=OP.mult)
                nc.vector.tensor_scalar(
                    nwt[:], nwt[:], -0.5, 1.5, op0=OP.mult, op1=OP.add
                )
                nc.vector.tensor_tensor(rstd4[:], rstd4[:], nwt[:], op=OP.mult)
            bt4 = cst.tile([128, NT], F32)
            nc.vector.tensor_tensor(
                bt4[:], mvall[:, 0 : 2 * NT : 2], rstd4[:], op=OP.mult
            )
            nc.vector.tensor_scalar(bt4[:], bt4[:], -1.0, None, op0=OP.mult)
            rstd_t = [rstd4[:, i : i + 1] for i in range(NT)]
            bt_t = [bt4[:, i : i + 1] for i in range(NT)]

            def preamble(i):
                """z -> zT -> projT -> lhs rows for tile i."""
                z = wk.tile([128, D], BF16, tag="z", name=f"z{i}")
                nc.gpsimd.tensor_scalar(
                    z[:], x_t[i], rstd_t[i], bt_t[i],
                    op0=OP.mult, op1=OP.add,
                )
                zt = wk.tile([128, 2, 128], BF16, tag="zt", name=f"zt{i}")
                for kc in range(2):
                    eng = nc.sync if kc == 0 else nc.scalar
                    eng.dma_start(
                        zt[:, kc, :], z[:, kc * 128 : (kc + 1) * 128],
                        transpose=True,
                    )
                ppj = psa.tile([128, 128], F32, tag="blk", name=f"ppj{i}")
                for pos in (0, 64):
                    for kc in range(2):
                        nc.tensor.matmul(
                            ppj[pos : pos + 16, :],
                            ppw_sb[:, kc, :], zt[:, kc, :],
                            start=(kc == 0), stop=(kc == 1),
                            tile_position=(0, pos),
                        )
                lhs = lhs_t[i]
                nc.vector.tensor_scalar(
                    lhs[0:16, :], ppj[0:16, :], b0t[0:16, :], None, op0=OP.add
                )
                nc.vector.tensor_scalar(
                    lhs[64:80, :], ppj[64:80, :], b0t[64:80, :], None,
                    op0=OP.add,
                )

            def a_block(i, g, maxs_c):
                """score matmuls (K=32, strip 0) + DVE max for block g."""
                pa = psa.tile([128, BLK], F32, tag="blk", name=f"pa{i}_{g}")
                for h in range(2):
                    sl = slice(g * BLK + h * 512, g * BLK + (h + 1) * 512)
                    nc.tensor.matmul(
                        pa[:, h * 512 : (h + 1) * 512],
                        lhs_t[i][0:32, :], em3b[0:32, sl],
                        start=True, stop=True, tile_position=(0, 0),
                    )
                nc.vector.tensor_reduce(
                    maxs_c[:, g : g + 1], pa[:], axis=AX, op=OP.max
                )

            def a_close(i, maxs_c):
                maxs = wk.tile([128, 1], F32, tag="maxs", bufs=2)
                nc.vector.tensor_reduce(maxs[:], maxs_c[:], axis=AX, op=OP.max)
                nbm = wk.tile([128, 1], F32, tag="nbm", bufs=2,
                              name=f"nbm{i}")
                nc.vector.tensor_scalar(
                    nbm[:], maxs[:], -BETA, None, op0=OP.mult
                )
                return nbm

            def b_block(i, g, nbm, vsum_c):
                """score+dL0 matmuls (K=33, rows 64:97) + ACT exp."""
                pb = psb.tile([128, BLK], F32, tag="blk", name=f"pb{i}_{g}")
                for h in range(2):
                    sl = slice(g * BLK + h * 512, g * BLK + (h + 1) * 512)
                    nc.tensor.matmul(
                        pb[:, h * 512 : (h + 1) * 512],
                        lhs_t[i][64:97, :], em3b[64:97, sl],
                        start=True, stop=True, tile_position=(64, 0),
                    )
                btrash = wk.tile([128, BLK], BF16, tag="btrash", bufs=1)
                nc.scalar.activation(
                    btrash[:], pb[:], AF.Exp, scale=BETA, bias=nbm[:],
                    accum_out=vsum_c[:, g : g + 1],
                )

            def b_close(i, vsum_c):
                nc.vector.tensor_reduce(
                    vsum_all[:, i : i + 1], vsum_c[:], axis=AX, op=OP.add
                )

            def l0_chunk(s, j, psl):
                """L0 matvec for codes [(4s+j)*1024, +1024) onto psum
                partition row 32j of slot s (fp8, x WSCALE)."""
                c = 4 * s + j
                r = 32 * j
                for h in range(2):
                    sl = slice(h * 512, (h + 1) * 512)
                    for kc in range(2):
                        nc.tensor.matmul(
                            psl[r : r + 1, sl],
                            mke_sb[:, kc : kc + 1],
                            w_t[c][:, kc, sl],
                            start=(kc == 0), stop=(kc == 1),
                            tile_position=(0, r),
                        )

            def l0_close(s, psl):
                """S0 partials + delta*L0 extraction for slot s.  Only
                psum rows {0,32,64,96} carry data; other partitions hold
                harmless garbage (engine APs cannot stride partitions,
                the DMA below can)."""
                l0sb = wk.tile([128, BLK], F32, tag="l0sb", name=f"l0sb{s}")
                nc.scalar.activation(
                    l0sb[:], psl[:], AF.Copy, scale=DELTA / WSCALE
                )
                nc.sync.dma_start(l0scr[s, :, :], l0sb[0:97:32, :])
                strash = wk.tile([128, BLK], BF16, tag="strash", bufs=1)
                nc.scalar.activation(
                    strash[:], psl[:], AF.Exp, scale=1.0 / WSCALE,
                    accum_out=s0p[:, s : s + 1],
                )

            # ------- tile 0 phase A with the L0 stream interleaved -------
            preamble(0)
            maxs_c0 = wk.tile([128, NBLK], F32, tag="maxc", bufs=2,
                              name="maxc0")
            psl0 = psb.tile([128, BLK], F32, tag="blk", name="psl0")
            psl1 = psb.tile([128, BLK], F32, tag="blk", name="psl1")
            a_block(0, 0, maxs_c0)
            a_block(0, 1, maxs_c0)
            for g in range(2, NBLK):
                a_block(0, g, maxs_c0)
                j = g - 2
                if j < 4:
                    l0_chunk(0, j, psl0)
                    if j == 3:
                        l0_close(0, psl0)
                else:
                    l0_chunk(1, j - 4, psl1)
            l0_chunk(1, 2, psl1)
            l0_chunk(1, 3, psl1)
            l0_close(1, psl1)
            # dram -> bf16 row 32 (+ copy at 96) via gpsimd casting DMA
            nc.gpsimd.dma_start(em3b[32:33, :], l0scr[:, :, :])
            nc.gpsimd.dma_start(em3b[96:97, :], l0scr[:, :, :])
            nbm_i = a_close(0, maxs_c0)

            # ------- steady-state slots -------
            for i in range(NT):
                vsum_c = wk.tile([128, NBLK], F32, tag="vsumc", bufs=2,
                                 name=f"vsumc{i}")
                if i + 1 < NT:
                    preamble(i + 1)
                    maxs_cn = wk.tile([128, NBLK], F32, tag="maxc", bufs=2,
                                      name=f"maxc{i+1}")
                    if i == 0:
                        # B(0) is gated by the L0 round-trip: run A(1) first
                        for g in range(NBLK):
                            a_block(1, g, maxs_cn)
                        for g in range(NBLK):
                            b_block(0, g, nbm_i, vsum_c)
                    else:
                        for g in range(NBLK):
                            b_block(i, g, nbm_i, vsum_c)
                            a_block(i + 1, g, maxs_cn)
                    b_close(i, vsum_c)
                    nbm_i = a_close(i + 1, maxs_cn)
                else:
                    for g in range(NBLK):
                        b_block(i, g, nbm_i, vsum_c)
                    b_close(i, vsum_c)

            # ------- finalize -------
            dl0_all = cst.tile([128, NT], F32)
            nc.scalar.activation(dl0_all[:], vsum_all[:], AF.Ln)
            numacc = cst.tile([128, NT], F32)
            nc.vector.tensor_tensor(
                numacc[:], dl0_all[:], m_sb[:], op=OP.mult
            )
            numcol = cst.tile([128, 1], F32)
            nc.vector.tensor_reduce(numcol[:], numacc[:], axis=AX, op=OP.add)
            ps2 = psa.tile([128, 1], F32, tag="blk", name="ps2")
            nc.tensor.matmul(
                ps2[0:1, :], numcol[:], ones128[:], start=True, stop=True
            )
            pout = cst.tile([128, 1], F32)
            nc.vector.tensor_copy(pout[0:1, :], ps2[0:1, :])
            nc.sync.dma_start(out[0:1, :], pout[0:1, :])
            for s in range(2):
                nc.sync.dma_start(
                    out[1 + 4 * s : 5 + 4 * s, :], s0p[0:97:32, s : s + 1]
                )

    nc.finalize()
    return nc


def _prep_in_maps(xs, pad_mask, masked_masks, ln_gamma, ln_beta, projection,
                  embeddings, top_n_out, mask_emb):
    xsf = np.ascontiguousarray(np.asarray(xs, np.float32).reshape(B * T, D))
    pmf = np.asarray(pad_mask).reshape(-1).astype(bool)
    mmf = np.asarray(masked_masks).reshape(-1).astype(bool)
    gam = np.asarray(ln_gamma, np.float32)
    bet = np.asarray(ln_beta, np.float32)
    P = np.asarray(projection, np.float32)
    emb = np.asarray(embeddings, np.float32)[0]          # [E, N]
    W = np.asarray(top_n_out, np.float32)[0]             # [D, N]
    me = np.asarray(mask_emb, np.float32)

    # weight-only preprocessing (layouts, dtype casts, gamma folding)
    emt = np.concatenate([emb, emb * emb], axis=0).astype(NP_BF16)  # [32, N]
    wmat = np.ascontiguousarray(
        (W * WSCALE).reshape(2, 128, N).transpose(1, 0, 2)).astype(NP_FP8)
    mke = np.ascontiguousarray(me.reshape(2, 128).T).astype(NP_FP8)
    ppf = gam[:, None] * P                               # [D, E]
    ppw = np.ascontiguousarray(
        ppf.reshape(2, 128, E).transpose(1, 0, 2)).astype(NP_BF16)
    b0v = np.ascontiguousarray((bet @ P).reshape(16, 1)).astype(np.float32)

    shared = {"emt": emt, "wmat": wmat, "mke": mke, "ppw": ppw, "b0v": b0v}

    sel = np.nonzero(pmf & mmf)[0]
    dev = sel[: NCORES * TOK]
    chunks = np.array_split(dev, NCORES)
    in_maps = []
    for c in range(NCORES):
        idx = chunks[c]
        n = len(idx)
        xs_c = np.zeros((TOK, D), np.float32)
        m_c = np.zeros((TOK,), np.uint8)
        if n:
            xs_c[:n] = xsf[idx]
            m_c[:n] = 1
        in_maps.append({"xs": xs_c, "msk": m_c, **shared})
    return in_maps


def _host_residual(xs, pad_mask, masked_masks, ln_gamma, ln_beta, projection,
                   embeddings, top_n_out, mask_emb):
    """Exact L0[target] sum for the <=0.5% of masked tokens that do not fit
    the static 8x512 device capacity (plus the total mask count)."""
    xsf = np.asarray(xs, np.float64).reshape(B * T, D)
    pmf = np.asarray(pad_mask).reshape(-1).astype(bool)
    mmf = np.asarray(masked_masks).reshape(-1).astype(bool)
    sel = np.nonzero(pmf & mmf)[0]
    cnt = float(len(sel))
    resid = sel[NCORES * TOK :]
    if len(resid) == 0:
        return 0.0, cnt
    x = xsf[resid]
    mu = x.mean(-1, keepdims=True)
    var = ((x - mu) ** 2).mean(-1, keepdims=True)
    h = (x - mu) / np.sqrt(var + EPS)
    h = h * np.asarray(ln_gamma, np.float64) + np.asarray(ln_beta, np.float64)
    proj = h @ np.asarray(projection, np.float64)
    emb = np.asarray(embeddings, np.float64)[0]
    score = proj @ emb - 0.5 * (emb * emb).sum(0)[None, :]
    tgt = np.argmax(score, axis=-1)
    W = np.asarray(top_n_out, np.float64)[0]
    l0t = np.asarray(mask_emb, np.float64) @ W[:, tgt]
    return float(l0t.sum()), cnt


def kernel(**inputs) -> np.ndarray:
    if "nc" not in _CACHE:
        _CACHE["nc"] = _build_bass()
    nc = _CACHE["nc"]
    in_maps = _prep_in_maps(**inputs)
    res = bass_utils.run_bass_kernel_spmd(nc, in_maps, core_ids=list(range(NCORES)))
    num = 0.0
    s0sum = None
    for r in res.results:
        o = r["out"].reshape(9)
        num += float(o[0]) / (BETA * DELTA)
        if s0sum is None:
            s0sum = float(np.sum(o[1:9]))
    resid_num, cnt = _host_residual(**inputs)
    num += resid_num
    loss = np.float32(np.log(s0sum) - num / cnt)
    return np.asarray(loss, np.float32)


# revision 15
# speedup vs baseline: 1.8618x; 1.1317x over previous
"""BestRQ loss kernel for 8 Trainium2 NeuronCores.

Math (exact reformulations of the reference):
  - loss = sum_t m_t*ce_t / (sum(m)*C), m = pad & masked, C = 1.
  - At masked tokens, masked_xs == mask_emb exactly, so logits_t == L0 :=
    mask_emb @ W (one shared [N] row), logsumexp(logits_t) == S0.
    => loss = S0 - (sum_t m_t * L0[target_t]) / sum(m).
  - target_t = argmax_n score_tn, score_tn = proj_t . emb_n - 0.5*|emb_n|^2.
  - L0[target_t] extracted without an argmax index:
        maxs_t = max_n score_tn                       (K=32 stream, DVE max)
        ln sum_n exp(beta*(score_tn + delta*L0_n - maxs_t)) ~= beta*delta*L0[target_t]
    (beta=2000 makes the softmax a near-exact argmax selector; near-ties
    contribute noise orders of magnitude below the loss scale).
  - Only masked tokens matter: host gathers them, 512/core on 8 cores
    (4 tiles of 128); the handful of leftover tokens (masked count mod
    4096) are folded in exactly on the host - they are <0.5% of the sum.

Schedule notes (engine-ordered, all matmuls bf16 except the fp8 L0
matvec whose x64 pre-scale is compensated in ACT scale factors):
  - sync DMA queue carries only small latency-critical transfers (xs,
    masks, weights, z transposes); the 2MB fp8 W stream and the bf16
    embedding rows go on the scalar hwdge queue.
  - All LN work (bn_stats + one Ln batch + one Exp batch) is hoisted
    before the main loop; the per-tile ln(vsum) is one batched Ln at the
    end => 3 ACT table loads total.
  - L0 = mask_emb @ W lands on 4 psum partition rows (0/32/64/96), its
    matmuls interleaved into tile 0's score stream; the S0 logsumexp
    partials and delta*L0 extraction run partition-parallel on ACT, and
    a DRAM round-trip + gpsimd cast-DMA plants the delta*L0 row of the
    K=33 stream.
  - Steady state slot i: ACT exps B(i) while DVE max-reduces A(i+1) and
    PE streams both; z-affine/bias small ops ride the idle GPSIMD.
"""

import numpy as np

try:
    import concourse.bass as bass  # noqa: F401
except ImportError:  # pragma: no cover
    import sys

    sys.path.insert(0, "/opt/trn_rl_repo")
    import concourse.bass as bass  # noqa: F401

import concourse.mybir as mybir
from concourse import bacc, bass_utils
from concourse.tile import TileContext

F32 = mybir.dt.float32
BF16 = mybir.dt.bfloat16
FP8 = mybir.dt.float8e4
U8 = mybir.dt.uint8
NP_BF16 = mybir.dt.np(BF16)
NP_FP8 = mybir.dt.np(FP8)

B, T, D, E, N = 16, 512, 256, 16, 8192
NCORES = 8
EPS = 1e-5
DELTA = 1e-2
BETA = 2000.0
WSCALE = 64.0   # fp8 pre-scale of W (compensated in ACT scale factors)

NT = 4          # token tiles per core
TOK = NT * 128  # 512 device tokens per core; leftovers go to the host
BLK = 1024      # psum block width (2 banks)
NBLK = N // BLK

_CACHE = {}


def _build_bass():
    nc = bacc.Bacc(
        "TRN2", target_bir_lowering=False, debug=False, num_devices=NCORES
    )
    xs = nc.dram_tensor("xs", [TOK, D], F32, kind="ExternalInput")
    msk = nc.dram_tensor("msk", [TOK], U8, kind="ExternalInput")
    emt = nc.dram_tensor("emt", [33, N], BF16, kind="ExternalInput")
    wmat = nc.dram_tensor("wmat", [128, 2, N], FP8, kind="ExternalInput")
    mke = nc.dram_tensor("mke", [128, 2], FP8, kind="ExternalInput")
    ppw = nc.dram_tensor("ppw", [128, 2, E], BF16, kind="ExternalInput")
    b0v = nc.dram_tensor("b0v", [16, 1], F32, kind="ExternalInput")
    out = nc.dram_tensor("out", [9, 1], F32, kind="ExternalOutput")
    l0scr = nc.dram_tensor("l0scr", [2, 4, BLK], F32, kind="Internal")

    AX = mybir.AxisListType.X
    OP = mybir.AluOpType
    AF = mybir.ActivationFunctionType

    with TileContext(nc) as tc:
        with (
            tc.tile_pool(name="cst", bufs=1) as cst,
            tc.tile_pool(name="wstg", bufs=2) as wstg,
            tc.tile_pool(name="xsp", bufs=1) as xsp,
            tc.tile_pool(name="wk", bufs=2) as wk,
            tc.tile_pool(name="psa", bufs=2, space="PSUM") as psa,
            tc.tile_pool(name="psb", bufs=2, space="PSUM") as psb,
        ):
            # ------- latency-critical DMAs first -------
            xall = xsp.tile([128, NT, D], F32)
            nc.sync.dma_start(
                xall[:], xs.rearrange("(i p) d -> p i d", p=128)
            )
            x_t = [xall[:, i, :] for i in range(NT)]
            ppw_sb = cst.tile([128, 2, E], BF16)
            nc.sync.dma_start(ppw_sb[:], ppw[:, :, :])
            mke_sb = cst.tile([128, 2], FP8)
            nc.sync.dma_start(mke_sb[:], mke[:, :])
            # em3b: rows 0:16 emb, 16:32 emb^2, row 32 zeros (the A stream
            # is K=33 with a zero weight row so it shares the 64x128 tile
            # mode with the B stream - avoiding PE tiling-mode drains -
            # without depending on the delta*L0 round-trip); rows 64:96
            # duplicate emb/emb^2 and row 96 gets delta*L0 later.
            em3b = cst.tile([128, N], BF16)
            nc.scalar.dma_start(em3b[0:33, :], emt[:, :])
            nc.scalar.dma_start(em3b[64:96, :], emt[0:32, :])
            # fp8 W stream on the sync queue behind the small transfers
            w_t = []
            for c in range(NBLK):
                wt = wstg.tile([128, 2, BLK], FP8, name=f"wt{c}", tag="wt",
                               bufs=NBLK)
                nc.sync.dma_start(wt[:], wmat[:, :, c * BLK : (c + 1) * BLK])
                w_t.append(wt)
            msk8 = cst.tile([128, NT], U8)
            nc.sync.dma_start(msk8[:], msk.rearrange("(a b) -> b a", b=128))
            b0t = cst.tile([128, 1], F32)
            nc.sync.dma_start(b0t[0:16, :], b0v[:, :])
            nc.sync.dma_start(b0t[64:80, :], b0v[:, :])

            # ------- PE warmup: dense full-array (K=128) matmuls to coax
            # the activity-based clock gate toward K=8/8 -------
            wl = cst.tile([128, 128], BF16)
            nc.vector.memset(wl[:], 0.01)
            wr = cst.tile([128, 512], BF16)
            nc.vector.memset(wr[:], 0.01)
            pwarm = psa.tile([128, 512], F32, tag="blk", name="pwarm")
            for _ in range(6):
                nc.tensor.matmul(
                    pwarm[:], wl[:], wr[:], start=True, stop=True,
                )

            # ------- constants -------
            epsb = cst.tile([128, 1], F32)
            nc.vector.memset(epsb[:], EPS)
            ones128 = cst.tile([128, 1], F32)
            nc.vector.memset(ones128[:], 1.0)
            m_sb = cst.tile([128, NT], F32)
            nc.vector.tensor_copy(m_sb[:], msk8[:])
            s0p = cst.tile([128, 2], F32)
            vsum_all = cst.tile([128, NT], F32)

            lhs_t = []
            for li in range(NT):
                lh = cst.tile([128, 128], BF16, name=f"lhs{li}")
                nc.vector.memset(lh[:], 0.0)
                # rows 0:16 / 64:80 hold -0.5 only until the projection
                # write lands (32-aligned partition bases only)
                nc.vector.memset(lh[0:32, :], -0.5)
                nc.vector.memset(lh[64:96, :], -0.5)
                nc.vector.memset(lh[96:97, :], 1.0)
                lhs_t.append(lh)

            # ------- hoisted LN stats; rstd via DVE Newton rsqrt (keeps
            # Ln/Exp activation-table loads off the startup path; sample
            # variance of 256 N(0,1) values is concentrated near 1 so 4
            # iterations from y0=1 converge; padded slots diverge
            # harmlessly and are masked) -------
            mvall = cst.tile([128, 2 * NT], F32)
            for i in range(NT):
                stats = wk.tile([128, 6], F32, tag="stats", bufs=NT)
                nc.vector.bn_stats(stats[:], x_t[i])
                nc.vector.bn_aggr(mvall[:, 2 * i : 2 * i + 2], stats[:])
            ve = cst.tile([128, NT], F32)
            nc.vector.tensor_scalar(
                ve[:], mvall[:, 1 : 2 * NT : 2], EPS, None, op0=OP.add
            )
            rstd4 = cst.tile([128, NT], F32)
            nc.vector.memset(rstd4[:], 1.0)
            nwt = cst.tile([128, NT], F32)
            for _ in range(4):
                nc.vector.tensor_tensor(nwt[:], rstd4[:], rstd4[:], op=OP.mult)
                nc.vector.tensor_tensor(nwt[:], nwt[:], ve[:], op=OP.mult)
                nc.vector.tensor_scalar(
                    nwt[:], nwt[:], -0.5, 1.5, op0=OP.mult, op1=OP.add
                )
                nc.vector.tensor_tensor(rstd4[:], rstd4[:], nwt[:], op=OP.mult)
            bt4 = cst.tile([128, NT], F32)
            nc.vector.tensor_tensor(
                bt4[:], mvall[:, 0 : 2 * NT : 2], rstd4[:], op=OP.mult
            )
            nc.vector.tensor_scalar(bt4[:], bt4[:], -1.0, None, op0=OP.mult)
            rstd_t = [rstd4[:, i : i + 1] for i in range(NT)]
            bt_t = [bt4[:, i : i + 1] for i in range(NT)]

            def preamble(i):
                """z -> zT -> projT -> lhs rows for tile i."""
                z = wk.tile([128, D], BF16, tag="z", name=f"z{i}")
                nc.gpsimd.tensor_scalar(
                    z[:], x_t[i], rstd_t[i], bt_t[i],
                    op0=OP.mult, op1=OP.add,
                )
                zt = wk.tile([128, 2, 128], BF16, tag="zt", name=f"zt{i}")
                for kc in range(2):
                    eng = nc.sync if kc == 0 else nc.scalar
                    eng.dma_start(
                        zt[:, kc, :], z[:, kc * 128 : (kc + 1) * 128],
                        transpose=True,
                    )
                ppj = psa.tile([128, 128], F32, tag="blk", name=f"ppj{i}")
                for pos in (0, 64):
                    for kc in range(2):
                        nc.tensor.matmul(
                            ppj[pos : pos + 16, :],
                            ppw_sb[:, kc, :], zt[:, kc, :],
                            start=(kc == 0), stop=(kc == 1),
                            tile_position=(0, pos),
                        )
                lhs = lhs_t[i]
                nc.vector.tensor_scalar(
                    lhs[0:16, :], ppj[0:16, :], b0t[0:16, :], None, op0=OP.add
                )
                nc.vector.tensor_scalar(
                    lhs[64:80, :], ppj[64:80, :], b0t[64:80, :], None,
                    op0=OP.add,
                )

            def a_block(i, g, maxs_c):
                """score matmuls (K=33 w/ zero row, array tile T0) + DVE
                max for block g."""
                pa = psa.tile([128, BLK], F32, tag="blk", name=f"pa{i}_{g}")
                for h in range(2):
                    sl = slice(g * BLK + h * 512, g * BLK + (h + 1) * 512)
                    nc.tensor.matmul(
                        pa[:, h * 512 : (h + 1) * 512],
                        lhs_t[i][0:33, :], em3b[0:33, sl],
                        start=True, stop=True, tile_position=(0, 0),
                    )
                nc.vector.tensor_reduce(
                    maxs_c[:, g : g + 1], pa[:], axis=AX, op=OP.max
                )

            def a_close(i, maxs_c):
                maxs = wk.tile([128, 1], F32, tag="maxs", bufs=2)
                nc.vector.tensor_reduce(maxs[:], maxs_c[:], axis=AX, op=OP.max)
                nbm = wk.tile([128, 1], F32, tag="nbm", bufs=2,
                              name=f"nbm{i}")
                nc.vector.tensor_scalar(
                    nbm[:], maxs[:], -BETA, None, op0=OP.mult
                )
                return nbm

            def b_block(i, g, nbm, vsum_c):
                """score+dL0 matmuls (K=33, rows 64:97) + ACT exp."""
                pb = psb.tile([128, BLK], F32, tag="blk", name=f"pb{i}_{g}")
                for h in range(2):
                    sl = slice(g * BLK + h * 512, g * BLK + (h + 1) * 512)
                    nc.tensor.matmul(
                        pb[:, h * 512 : (h + 1) * 512],
                        lhs_t[i][64:97, :], em3b[64:97, sl],
                        start=True, stop=True, tile_position=(64, 0),
                    )
                btrash = wk.tile([128, BLK], BF16, tag="btrash", bufs=1)
                nc.scalar.activation(
                    btrash[:], pb[:], AF.Exp, scale=BETA, bias=nbm[:],
                    accum_out=vsum_c[:, g : g + 1],
                )

            def b_close(i, vsum_c):
                nc.vector.tensor_reduce(
                    vsum_all[:, i : i + 1], vsum_c[:], axis=AX, op=OP.add
                )

            def l0_slot(s, psl):
                """L0 matvec for codes [4096s, +4096) onto psum partition
                rows {0,32,64,96} of slot s (fp8, x WSCALE).  The four
                rows are four independent column tiles of the array -
                matmuls interleaved across j run concurrently."""
                for h in range(2):
                    sl = slice(h * 512, (h + 1) * 512)
                    for kc in range(2):
                        for j in range(4):
                            nc.tensor.matmul(
                                psl[32 * j : 32 * j + 1, sl],
                                mke_sb[:, kc : kc + 1],
                                w_t[4 * s + j][:, kc, sl],
                                start=(kc == 0), stop=(kc == 1),
                                tile_position=(0, 32 * j),
                            )

            def l0_close(s, psl):
                """S0 partials + delta*L0 extraction for slot s.  Only
                psum rows {0,32,64,96} carry data; other partitions hold
                harmless garbage (engine APs cannot stride partitions,
                the DMA below can)."""
                l0sb = wk.tile([128, BLK], F32, tag="l0sb", name=f"l0sb{s}")
                nc.scalar.activation(
                    l0sb[:], psl[:], AF.Copy, scale=DELTA / WSCALE
                )
                nc.sync.dma_start(l0scr[s, :, :], l0sb[0:97:32, :])
                strash = wk.tile([128, BLK], BF16, tag="strash", bufs=1)
                nc.scalar.activation(
                    strash[:], psl[:], AF.Exp, scale=1.0 / WSCALE,
                    accum_out=s0p[:, s : s + 1],
                )

            # ------- tile 0 phase A with the L0 slots grouped in (mode
            # switches on the PE array are drains - keep mode-mates
            # contiguous) -------
            preamble(0)
            maxs_c0 = wk.tile([128, NBLK], F32, tag="maxc", bufs=2,
                              name="maxc0")
            psl0 = psb.tile([128, BLK], F32, tag="blk", name="psl0")
            psl1 = psb.tile([128, BLK], F32, tag="blk", name="psl1")
            for g in range(5):
                a_block(0, g, maxs_c0)
            l0_slot(0, psl0)
            l0_close(0, psl0)
            for g in range(5, NBLK):
                a_block(0, g, maxs_c0)
            l0_slot(1, psl1)
            l0_close(1, psl1)
            # dram -> bf16 row 96 via gpsimd casting DMA
            nc.gpsimd.dma_start(em3b[96:97, :], l0scr[:, :, :])
            nbm_i = a_close(0, maxs_c0)

            # ------- steady-state slots -------
            for i in range(NT):
                vsum_c = wk.tile([128, NBLK], F32, tag="vsumc", bufs=2,
                                 name=f"vsumc{i}")
                if i + 1 < NT:
                    preamble(i + 1)
                    maxs_cn = wk.tile([128, NBLK], F32, tag="maxc", bufs=2,
                                      name=f"maxc{i+1}")
                    if i == 0:
                        # B(0) is gated by the L0 round-trip: run A(1) first
                        for g in range(NBLK):
                            a_block(1, g, maxs_cn)
                        for g in range(NBLK):
                            b_block(0, g, nbm_i, vsum_c)
                    else:
                        for g in range(NBLK):
                            b_block(i, g, nbm_i, vsum_c)
                            a_block(i + 1, g, maxs_cn)
                    b_close(i, vsum_c)
                    nbm_i = a_close(i + 1, maxs_cn)
                else:
                    for g in range(NBLK):
                        b_block(i, g, nbm_i, vsum_c)
                    b_close(i, vsum_c)

            # ------- finalize -------
            dl0_all = cst.tile([128, NT], F32)
            nc.scalar.activation(dl0_all[:], vsum_all[:], AF.Ln)
            numacc = cst.tile([128, NT], F32)
            nc.vector.tensor_tensor(
                numacc[:], dl0_all[:], m_sb[:], op=OP.mult
            )
            numcol = cst.tile([128, 1], F32)
            nc.vector.tensor_reduce(numcol[:], numacc[:], axis=AX, op=OP.add)
            ps2 = psa.tile([128, 1], F32, tag="blk", name="ps2")
            nc.tensor.matmul(
                ps2[0:1, :], numcol[:], ones128[:], start=True, stop=True
            )
            pout = cst.tile([128, 1], F32)
            nc.vector.tensor_copy(pout[0:1, :], ps2[0:1, :])
            nc.sync.dma_start(out[0:1, :], pout[0:1, :])
            for s in range(2):
                nc.sync.dma_start(
                    out[1 + 4 * s : 5 + 4 * s, :], s0p[0:97:32, s : s + 1]
                )

    nc.finalize()
    return nc


def _prep_in_maps(xs, pad_mask, masked_masks, ln_gamma, ln_beta, projection,
                  embeddings, top_n_out, mask_emb):
    xsf = np.ascontiguousarray(np.asarray(xs, np.float32).reshape(B * T, D))
    pmf = np.asarray(pad_mask).reshape(-1).astype(bool)
    mmf = np.asarray(masked_masks).reshape(-1).astype(bool)
    gam = np.asarray(ln_gamma, np.float32)
    bet = np.asarray(ln_beta, np.float32)
    P = np.asarray(projection, np.float32)
    emb = np.asarray(embeddings, np.float32)[0]          # [E, N]
    W = np.asarray(top_n_out, np.float32)[0]             # [D, N]
    me = np.asarray(mask_emb, np.float32)

    # weight-only preprocessing (layouts, dtype casts, gamma folding)
    emt = np.concatenate(
        [emb, emb * emb, np.zeros((1, N), np.float32)], axis=0
    ).astype(NP_BF16)                                    # [33, N]
    wmat = np.ascontiguousarray(
        (W * WSCALE).reshape(2, 128, N).transpose(1, 0, 2)).astype(NP_FP8)
    mke = np.ascontiguousarray(me.reshape(2, 128).T).astype(NP_FP8)
    ppf = gam[:, None] * P                               # [D, E]
    ppw = np.ascontiguousarray(
        ppf.reshape(2, 128, E).transpose(1, 0, 2)).astype(NP_BF16)
    b0v = np.ascontiguousarray((bet @ P).reshape(16, 1)).astype(np.float32)

    shared = {"emt": emt, "wmat": wmat, "mke": mke, "ppw": ppw, "b0v": b0v}

    sel = np.nonzero(pmf & mmf)[0]
    dev = sel[: NCORES * TOK]
    chunks = np.array_split(dev, NCORES)
    in_maps = []
    for c in range(NCORES):
        idx = chunks[c]
        n = len(idx)
        xs_c = np.zeros((TOK, D), np.float32)
        m_c = np.zeros((TOK,), np.uint8)
        if n:
            xs_c[:n] = xsf[idx]
            m_c[:n] = 1
        in_maps.append({"xs": xs_c, "msk": m_c, **shared})
    return in_maps


def _host_residual(xs, pad_mask, masked_masks, ln_gamma, ln_beta, projection,
                   embeddings, top_n_out, mask_emb):
    """Exact L0[target] sum for the <=0.5% of masked tokens that do not fit
    the static 8x512 device capacity (plus the total mask count)."""
    xsf = np.asarray(xs, np.float64).reshape(B * T, D)
    pmf = np.asarray(pad_mask).reshape(-1).astype(bool)
    mmf = np.asarray(masked_masks).reshape(-1).astype(bool)
    sel = np.nonzero(pmf & mmf)[0]
    cnt = float(len(sel))
    resid = sel[NCORES * TOK :]
    if len(resid) == 0:
        return 0.0, cnt
    x = xsf[resid]
    mu = x.mean(-1, keepdims=True)
    var = ((x - mu) ** 2).mean(-1, keepdims=True)
    h = (x - mu) / np.sqrt(var + EPS)
    h = h * np.asarray(ln_gamma, np.float64) + np.asarray(ln_beta, np.float64)
    proj = h @ np.asarray(projection, np.float64)
    emb = np.asarray(embeddings, np.float64)[0]
    score = proj @ emb - 0.5 * (emb * emb).sum(0)[None, :]
    tgt = np.argmax(score, axis=-1)
    W = np.asarray(top_n_out, np.float64)[0]
    l0t = np.asarray(mask_emb, np.float64) @ W[:, tgt]
    return float(l0t.sum()), cnt


def kernel(**inputs) -> np.ndarray:
    if "nc" not in _CACHE:
        _CACHE["nc"] = _build_bass()
    nc = _CACHE["nc"]
    in_maps = _prep_in_maps(**inputs)
    res = bass_utils.run_bass_kernel_spmd(nc, in_maps, core_ids=list(range(NCORES)))
    num = 0.0
    s0sum = None
    for r in res.results:
        o = r["out"].reshape(9)
        num += float(o[0]) / (BETA * DELTA)
        if s0sum is None:
            s0sum = float(np.sum(o[1:9]))
    resid_num, cnt = _host_residual(**inputs)
    num += resid_num
    loss = np.float32(np.log(s0sum) - num / cnt)
    return np.asarray(loss, np.float32)
